# revision 1
# baseline (speedup 1.0000x reference)
"""Trainium2 Bass kernel for the 2-layer S4D block (nn_MetaS4History).

Strategy (8 cores, single launch):
  - Conv phases H-sharded (64 channels/core, full batch): chunked-SSD convolution
    with T=128 chunks: per-h matmuls (G-build, intra, injection) + a 16-step
    DVE scan for inter-chunk states.
  - GLU projections B-sharded (2 batch elems/core, full H): big shared-weight
    matmuls.
  - Phase boundaries resharded with AllToAll collectives (3 total).
All compute in fp32 on device; host does only layout transforms.
"""
import contextlib
import numpy as np
import concourse.bacc as bacc
import concourse.mybir as mybir
from concourse.tile import TileContext
from concourse.bass_utils import run_bass_kernel_spmd

F32 = mybir.dt.float32
AF = mybir.ActivationFunctionType
OP = mybir.AluOpType

CORES = 8
B, L, H, N = 16, 2048, 512, 64
T, C = 128, 16          # chunk len, chunk count
HS = H // CORES         # 64 channels per core
B2 = B // CORES         # 2 batch per core (GLU phase)
NHB = HS // 2           # 32 h-blocks (h = 2*hblk + hpar)
RG = [list(range(CORES))]

_NC_CACHE = {}


def _build_table(eng, tre, tim, seed_re, seed_im, mul_re, mul_im, wk, nhb):
    """Power table via doubling: tab[:, hb, j] = seed * mul^j, j in 0..T-1.
    tre/tim: [128, nhb*T] tiles; seed/mul: [128, nhb] APs (sliced); wk: pool."""
    t3re = tre[:].rearrange("p (h j) -> p h j", j=T)
    t3im = tim[:].rearrange("p (h j) -> p h j", j=T)
    eng.tensor_copy(t3re[:, :, 0:1], seed_re.unsqueeze(2))
    eng.tensor_copy(t3im[:, :, 0:1], seed_im.unsqueeze(2))
    mre = wk.tile([128, nhb], F32, tag="dbl_mre", name="dbl_mre")
    mim = wk.tile([128, nhb], F32, tag="dbl_mim", name="dbl_mim")
    q1 = wk.tile([128, nhb], F32, tag="dbl_q1", name="dbl_q1")
    q2 = wk.tile([128, nhb], F32, tag="dbl_q2", name="dbl_q2")
    sc1 = wk.tile([128, nhb * T // 2], F32, tag="dbl_s1", name="dbl_s1")
    eng.tensor_copy(mre[:], mul_re)
    eng.tensor_copy(mim[:], mul_im)
    m = 1
    while m < T:
        mbre = mre[:].unsqueeze(2).broadcast_to([128, nhb, m])
        mbim = mim[:].unsqueeze(2).broadcast_to([128, nhb, m])
        s1 = sc1[:].rearrange("p (h j) -> p h j", j=T // 2)[:, :, 0:m]
        src_re, src_im = t3re[:, :, 0:m], t3im[:, :, 0:m]
        dst_re, dst_im = t3re[:, :, m : 2 * m], t3im[:, :, m : 2 * m]
        eng.tensor_mul(s1, src_im, mbim)
        eng.tensor_mul(dst_re, src_re, mbre)
        eng.tensor_sub(dst_re, dst_re, s1)
        eng.tensor_mul(s1, src_im, mbre)
        eng.tensor_mul(dst_im, src_re, mbim)
        eng.tensor_add(dst_im, dst_im, s1)
        m *= 2
        if m < T:
            eng.tensor_mul(q1[:], mre[:], mre[:])
            eng.tensor_mul(q2[:], mim[:], mim[:])
            eng.tensor_mul(mim[:], mre[:], mim[:])
            eng.tensor_add(mim[:], mim[:], mim[:])
            eng.tensor_sub(mre[:], q1[:], q2[:])


def build_kernel(debug=False):
    key = debug
    if key in _NC_CACHE:
        return _NC_CACHE[key]
    nc = bacc.Bacc(num_devices=CORES)
    v = nc.vector
    gp = nc.gpsimd
    sc = nc.scalar
    te = nc.tensor

    # ---------------- DRAM I/O ----------------
    u0_in = nc.dram_tensor("u0", [T, B, C, HS], F32, kind="ExternalInput")
    u0b_in = nc.dram_tensor("u0b", [H, B2, L], F32, kind="ExternalInput")
    trimask_in = nc.dram_tensor("trimask", [T, T], F32, kind="ExternalInput")
    ident_in = nc.dram_tensor("ident", [T, T], F32, kind="ExternalInput")
    par_in = {}
    for l in (0, 1):
        for nm in ("ldt", "lare", "aim", "cre", "cim"):
            par_in[(nm, l)] = nc.dram_tensor(f"{nm}{l}", [128, NHB], F32, kind="ExternalInput")
        par_in[("drep", l)] = nc.dram_tensor(f"drep{l}", [128, HS], F32, kind="ExternalInput")
        par_in[("wt", l)] = nc.dram_tensor(f"wt{l}", [H, 2 * H], F32, kind="ExternalInput")
    brep0_in = nc.dram_tensor("brep0", [128, 8], F32, kind="ExternalInput")
    b1row_in = nc.dram_tensor("b1row", [1, 2 * H], F32, kind="ExternalInput")

    a2a_y_in = [nc.dram_tensor(f"a2aY{l}_in", [CORES, HS, B2, L], F32) for l in (0, 1)]
    a2a_y_out = [nc.dram_tensor(f"a2aY{l}_out", [CORES, HS, B2, L], F32) for l in (0, 1)]
    a2a_u_in = nc.dram_tensor("a2aU_in", [CORES, T, B2, C, HS], F32)
    a2a_u_out = nc.dram_tensor("a2aU_out", [CORES, T, B2, C, HS], F32)
    out_z = nc.dram_tensor("out", [B2, L, H], F32, kind="ExternalOutput")
    dbg = {}
    if debug:
        dbg["yact0"] = nc.dram_tensor("dbg_yact0", [CORES, HS, B2, L], F32, kind="ExternalOutput")
        dbg["u1"] = nc.dram_tensor("dbg_u1", [T, B, C, HS], F32, kind="ExternalOutput")

    with TileContext(nc) as tc, contextlib.ExitStack() as top:
        cpool = top.enter_context(tc.tile_pool(name="consts", bufs=1))
        trimask = cpool.tile([T, T], F32, tag="trimask", name="trimask")
        ident = cpool.tile([T, T], F32, tag="ident", name="ident")
        nc.sync.dma_start(trimask[:], trimask_in[:])
        nc.sync.dma_start(ident[:], ident_in[:])
        csts = cpool.tile([128, 32], F32, tag="csts", name="csts")
        SINC = [1.0, -1.0 / 6, 1.0 / 120, -1.0 / 5040, 1.0 / 362880, -1.0 / 39916800]
        COSC = [1.0, -1.0 / 2, 1.0 / 24, -1.0 / 720, 1.0 / 40320, -1.0 / 3628800]
        for k in range(6):
            nc.any.memset(csts[:, k : k + 1], SINC[k])
            nc.any.memset(csts[:, 6 + k : 7 + k], COSC[k])
        nc.any.memset(csts[:, 12:13], -1.0)
        nc.any.memset(csts[:, 13:14], 2.0)
        nc.any.memset(csts[:, 14:15], 1.0 / 16)
        import math
        for k in range(11):
            nc.any.memset(csts[:, 16 + k : 17 + k], 1.0 / math.factorial(k))
        nc.any.memset(csts[:, 27:28], 1.0 / 8)
        brep0 = cpool.tile([128, 8], F32, tag="brep0", name="brep0")
        nc.sync.dma_start(brep0[:], brep0_in[:])

        upool = top.enter_context(tc.tile_pool(name="u", bufs=1))
        u_sb = upool.tile([T, B * C * HS], F32, tag="u_sb", name="u_sb")  # [j,(b,c,h)]
        nc.sync.dma_start(u_sb[:], u0_in[:].rearrange("j b c h -> j (b c h)"))

        def u_slice(h, bq=None):
            b0, nb = (0, B) if bq is None else (bq * 8, 8)
            ap = u_sb[:].rearrange("j (b c h) -> j b c h", b=B, c=C)
            return ap[:, b0 : b0 + nb, :, h]

        for l in (0, 1):
            # ======== CONV PHASE (H-shard) ========
            with contextlib.ExitStack() as cv:
                pp = cv.enter_context(tc.tile_pool(name=f"par{l}", bufs=1))
                P = {}
                for nm in ("ldt", "lare", "aim", "cre", "cim"):
                    P[nm] = pp.tile([128, NHB], F32, tag=f"p_{nm}", name=f"p_{nm}")
                    nc.sync.dma_start(P[nm][:], par_in[(nm, l)][:])
                drep = pp.tile([128, HS], F32, tag="p_drep", name="p_drep")
                nc.sync.dma_start(drep[:], par_in[("drep", l)][:])

                def wk(tag):
                    return pp.tile([128, NHB], F32, tag=tag, name=tag)[:]

                neg1 = csts[:, 12:13]
                two = csts[:, 13:14]
                s16 = csts[:, 14:15]

                def exp_poly(out, x):
                    """out = e^x via (T10(x/8))^8; |x| <= 8. Accurate to ~1e-7."""
                    ea = wk("exp_a")
                    et = wk("exp_t")
                    v.tensor_scalar(ea, x, csts[:, 27:28], None, op0=OP.mult)  # y = x/8
                    v.tensor_scalar(et, ea, csts[:, 26:27], csts[:, 25:26], op0=OP.mult, op1=OP.add)
                    for k in range(8, -1, -1):
                        v.tensor_mul(et, et, ea)
                        v.tensor_scalar(et, et, csts[:, 16 + k : 17 + k], None, op0=OP.add)
                    for _ in range(3):
                        v.tensor_mul(et, et, et)
                    v.tensor_copy(out, et)

                dt, eA = wk("dt"), wk("eA")
                exp_poly(dt, P["ldt"][:])
                exp_poly(eA, P["lare"][:])
                dtAre, dtAim = wk("dtAre"), wk("dtAim")
                v.scalar_tensor_tensor(dtAre, dt, -1.0, eA, op0=OP.mult, op1=OP.mult)
                v.tensor_mul(dtAim, dt, P["aim"][:])
                mag = wk("mag")
                exp_poly(mag, dtAre)
                q, x2 = wk("q"), wk("x2")
                v.tensor_scalar(q, dtAim, s16, None, op0=OP.mult)
                v.tensor_mul(x2, q, q)
                acc, t1, t2 = wk("acc"), wk("t1"), wk("t2")
                cr, ci = wk("cr"), wk("ci")
                v.tensor_scalar(acc, x2, csts[:, 5:6], csts[:, 4:5], op0=OP.mult, op1=OP.add)
                for k in (3, 2, 1, 0):
                    v.tensor_mul(t1, acc, x2)
                    v.tensor_scalar(acc, t1, csts[:, k : k + 1], None, op0=OP.add)
                v.tensor_mul(ci, acc, q)
                v.tensor_scalar(acc, x2, csts[:, 11:12], csts[:, 10:11], op0=OP.mult, op1=OP.add)
                for k in (9, 8, 7, 6):
                    v.tensor_mul(t1, acc, x2)
                    v.tensor_scalar(acc, t1, csts[:, k : k + 1], None, op0=OP.add)
                v.tensor_copy(cr, acc)
                for _ in range(4):
                    v.tensor_mul(t1, cr, cr)
                    v.tensor_mul(t2, ci, ci)
                    v.scalar_tensor_tensor(acc, cr, 2.0, ci, op0=OP.mult, op1=OP.mult)
                    v.tensor_sub(cr, t1, t2)
                    v.tensor_copy(ci, acc)
                wre, wim = wk("wre"), wk("wim")
                v.tensor_mul(wre, mag, cr)
                v.tensor_mul(wim, mag, ci)
                m2, im2 = wk("m2"), wk("im2")
                v.tensor_mul(m2, mag, mag)
                v.reciprocal(im2, m2)
                rpre, rpim = wk("rpre"), wk("rpim")
                v.tensor_mul(rpre, wre, im2)
                v.tensor_mul(rpim, wim, im2)
                wm1re = wk("wm1re")
                v.tensor_scalar(wm1re, wre, neg1, None, op0=OP.add)
                tre, tim = wk("tre"), wk("tim")
                v.tensor_mul(t1, P["cre"][:], wm1re)
                v.tensor_mul(t2, P["cim"][:], wim)
                v.tensor_sub(tre, t1, t2)
                v.tensor_mul(t1, P["cre"][:], wim)
                v.tensor_mul(t2, P["cim"][:], wm1re)
                v.tensor_add(tim, t1, t2)
                den, invd = wk("den"), wk("invd")
                v.tensor_mul(t1, eA, eA)
                v.tensor_mul(t2, P["aim"][:], P["aim"][:])
                v.tensor_add(den, t1, t2)
                v.reciprocal(invd, den)
                ccr, cci = wk("ccr"), wk("cci")
                v.tensor_mul(t1, tre, eA)
                v.tensor_mul(t2, tim, P["aim"][:])
                v.tensor_sub(acc, t2, t1)
                v.tensor_mul(ccr, acc, invd)
                v.tensor_mul(t1, tre, P["aim"][:])
                v.tensor_mul(t2, tim, eA)
                v.tensor_add(acc, t1, t2)
                v.tensor_mul(t1, acc, invd)
                v.tensor_scalar(cci, t1, neg1, None, op0=OP.mult)
                esr, esi = wk("esr"), wk("esi")
                v.tensor_mul(t1, ccr, wre)
                v.tensor_mul(t2, cci, wim)
                v.tensor_sub(acc, t1, t2)
                v.tensor_scalar(esr, acc, two, None, op0=OP.mult)
                v.tensor_mul(t1, ccr, wim)
                v.tensor_mul(t2, cci, wre)
                v.tensor_add(acc, t1, t2)
                v.tensor_scalar(esi, acc, two, None, op0=OP.mult)
                wtr, wti = wk("wtr"), wk("wti")
                v.tensor_copy(wtr, wre)
                v.tensor_copy(wti, wim)
                for _ in range(7):
                    v.tensor_mul(t1, wtr, wtr)
                    v.tensor_mul(t2, wti, wti)
                    v.scalar_tensor_tensor(acc, wtr, 2.0, wti, op0=OP.mult, op1=OP.mult)
                    v.tensor_sub(wtr, t1, t2)
                    v.tensor_copy(wti, acc)
                dre, dim_ = wk("dre"), wk("dim")
                v.tensor_copy(dre, wtr)
                v.tensor_scalar(dim_, wti, neg1, None, op0=OP.mult)

                # ---------- conv machinery, split in hblk halves ----------
                gt_pool = cv.enter_context(tc.tile_pool(name=f"gt{l}", bufs=3))
                ya_pool = cv.enter_context(tc.tile_pool(name=f"ya{l}", bufs=2))
                HG = 8
                NHB2 = NHB // 2
                for half in (0, 1):
                  hb0 = half * NHB2
                  with contextlib.ExitStack() as hsc:
                    tpr = hsc.enter_context(tc.tile_pool(name=f"tabR{l}{half}", bufs=1))
                    Rp_re = tpr.tile([128, NHB2 * T], F32, tag="Rp_re", name="Rp_re")
                    Rp_im = tpr.tile([128, NHB2 * T], F32, tag="Rp_im", name="Rp_im")
                    with tc.tile_pool(name=f"dblR{l}{half}", bufs=1) as dwk:
                        _build_table(gp, Rp_re, Rp_im,
                                     rpre[:, hb0 : hb0 + NHB2], rpim[:, hb0 : hb0 + NHB2],
                                     rpre[:, hb0 : hb0 + NHB2], rpim[:, hb0 : hb0 + NHB2],
                                     dwk, NHB2)

                    stp = hsc.enter_context(tc.tile_pool(name=f"st{l}{half}", bufs=1))
                    X_re = stp.tile([128, NHB2 * B * C], F32, tag="X_re", name="X_re")
                    X_im = stp.tile([128, NHB2 * B * C], F32, tag="X_im", name="X_im")
                    X_re4 = X_re[:].rearrange("p (h b c) -> p h b c", b=B, c=C)
                    X_im4 = X_im[:].rearrange("p (h b c) -> p h b c", b=B, c=C)

                    # collection (transpose R' slices on the fly)
                    with tc.tile_pool(name=f"wsl{l}{half}", bufs=3) as wslp, \
                         tc.tile_pool(name=f"pst{l}{half}", bufs=2, space="PSUM") as pstp, \
                         tc.tile_pool(name=f"psc{l}{half}", bufs=2, space="PSUM") as pscp:
                        for k in range(NHB2):
                            wsl = [wslp.tile([128, T], F32, tag=f"wsl{comp}", name=f"wsl{comp}")
                                   for comp in (0, 1)]
                            for comp, Rt in enumerate((Rp_re, Rp_im)):
                                psT = pstp.tile([128, T], F32, tag="psT", name="psT")
                                te.transpose(psT[:], Rt[:, k * T : (k + 1) * T], ident[:])
                                sc.activation(wsl[comp][:], psT[:], AF.Copy)
                            psr = pscp.tile([128, B * C], F32, tag="psr", name="psr")
                            psi = pscp.tile([128, B * C], F32, tag="psi", name="psi")
                            for hp in (0, 1):
                                h = 2 * (hb0 + k) + hp
                                us = u_slice(h)
                                te.matmul(psr[64 * hp : 64 * hp + 64, :],
                                          wsl[0][:, 64 * hp : 64 * hp + 64], us, start=True, stop=True)
                                te.matmul(psi[64 * hp : 64 * hp + 64, :],
                                          wsl[1][:, 64 * hp : 64 * hp + 64], us, start=True, stop=True)
                            sc.activation(X_re4[:, k, :, :], psr[:], AF.Copy)
                            sc.activation(X_im4[:, k, :, :], psi[:], AF.Copy)

                    # scan (in place: X becomes Sacc)
                    with tc.tile_pool(name=f"scan{l}{half}", bufs=1) as sp:
                        def stile(nm):
                            return sp.tile([128, NHB2 * B], F32, tag=nm, name=nm)[:].rearrange(
                                "p (h b) -> p h b", b=B)
                        Sr3, Si3 = stile("Sr"), stile("Si")
                        t_r3, t_i3 = stile("tm_r"), stile("tm_i")
                        w13, w23 = stile("w1"), stile("w2")
                        nc.any.memset(Sr3, 0.0)
                        nc.any.memset(Si3, 0.0)
                        dreb = dre[:, hb0 : hb0 + NHB2].unsqueeze(2).broadcast_to([128, NHB2, B])
                        dimb = dim_[:, hb0 : hb0 + NHB2].unsqueeze(2).broadcast_to([128, NHB2, B])
                        for ccc in range(C):
                            xr, xi = X_re4[:, :, :, ccc], X_im4[:, :, :, ccc]
                            v.tensor_add(t_r3, Sr3, xr)
                            gp.tensor_add(t_i3, Si3, xi)
                            sc.activation(xr, Sr3, AF.Copy)
                            sc.activation(xi, Si3, AF.Copy)
                            v.tensor_mul(w13, t_r3, dreb)
                            v.tensor_mul(w23, t_i3, dimb)
                            v.tensor_sub(Sr3, w13, w23)
                            v.tensor_mul(w13, t_i3, dreb)
                            v.tensor_mul(w23, t_r3, dimb)
                            v.tensor_add(Si3, w13, w23)

                    # E table for this half
                    gp_ps = hsc.enter_context(tc.tile_pool(name=f"gps{l}{half}", bufs=2, space="PSUM"))
                    cv_ps = hsc.enter_context(tc.tile_pool(name=f"cvps{l}{half}", bufs=2, space="PSUM"))
                    tpe = hsc.enter_context(tc.tile_pool(name=f"tabE{l}{half}", bufs=1))
                    E_re = tpe.tile([128, NHB2 * T], F32, tag="E_re", name="E_re")
                    E_im = tpe.tile([128, NHB2 * T], F32, tag="E_im", name="E_im")
                    with tc.tile_pool(name=f"dblE{l}{half}", bufs=1) as dwk:
                        _build_table(v, E_re, E_im,
                                     esr[:, hb0 : hb0 + NHB2], esi[:, hb0 : hb0 + NHB2],
                                     wre[:, hb0 : hb0 + NHB2], wim[:, hb0 : hb0 + NHB2],
                                     dwk, NHB2)

                    # per-h conv
                    yg = [None, None]
                    for hh in range(HS // 2):
                        h = 2 * hb0 + hh
                        hp, hb = h & 1, h >> 1
                        base = 64 * hp
                        kb = hb - hb0
                        er = E_re[base : base + 64, kb * T : (kb + 1) * T]
                        ei = E_im[base : base + 64, kb * T : (kb + 1) * T]
                        rr = Rp_re[base : base + 64, kb * T : (kb + 1) * T]
                        ri = Rp_im[base : base + 64, kb * T : (kb + 1) * T]
                        psG = gp_ps.tile([128, T], F32, tag="psG", name="psG")
                        te.matmul(psG[:], rr, er, start=True, stop=False)
                        te.matmul(psG[:], ri, ei, start=False, stop=True)
                        GTt = gt_pool.tile([128, T], F32, tag="GTt", name="GTt")
                        GT = gt_pool.tile([128, T], F32, tag="GT", name="GT")
                        v.tensor_mul(GTt[:], psG[:], trimask[:])
                        v.scalar_tensor_tensor(GT[:], ident[:], drep[:, h : h + 1], GTt[:],
                                               op0=OP.mult, op1=OP.add)
                        if hh % HG == 0:
                            yg = [ya_pool.tile([128, HG * T], F32, tag=f"yg{qq}", name=f"yg{qq}")
                                  for qq in (0, 1)]
                        for qq in (0, 1):
                            ps = cv_ps.tile([128, T], F32, tag="ps", name="ps")
                            lu = u_slice(h, qq)
                            te.matmul(ps[:], lu, GT[:], start=True, stop=False)
                            lr = X_re[base : base + 64,
                                      kb * B * C + qq * 128 : kb * B * C + qq * 128 + 128]
                            li = X_im[base : base + 64,
                                      kb * B * C + qq * 128 : kb * B * C + qq * 128 + 128]
                            te.matmul(ps[:], lr, er, start=False, stop=False)
                            te.matmul(ps[:], li, ei, start=False, stop=True)
                            sc.activation(yg[qq][:, (hh % HG) * T : (hh % HG + 1) * T], ps[:],
                                          AF.Gelu_apprx_tanh)
                        if hh % HG == HG - 1:
                            hg0 = h - HG + 1
                            for qq in (0, 1):
                                ygv = yg[qq][:].rearrange("bc (hh2 j) -> bc hh2 j", j=T)
                                for dd in range(4):
                                    d = qq * 4 + dd
                                    dst = a2a_y_in[l][d, hg0 : hg0 + HG, :, :].rearrange(
                                        "hh2 b2 (c j) -> (b2 c) hh2 j", j=T)
                                    nc.sync.dma_start(dst, ygv[32 * dd : 32 * dd + 32, :, :])

            # ======== AllToAll y ========
            gp.collective_compute(
                "AllToAll", OP.bypass, replica_groups=RG,
                ins=[a2a_y_in[l][:].opt()], outs=[a2a_y_out[l][:].opt()])

            # ======== GLU PHASE (B-shard) ========
            with contextlib.ExitStack() as gl:
                gpool = gl.enter_context(tc.tile_pool(name=f"glu{l}", bufs=1))
                wtiles = [gpool.tile([128, 2 * H], F32, tag=f"wt{k}", name=f"wt{k}") for k in range(4)]
                ytiles = [gpool.tile([128, B2 * L], F32, tag=f"yk{k}", name=f"yk{k}") for k in range(4)]
                for kt in range(4):
                    nc.sync.dma_start(wtiles[kt][:], par_in[("wt", l)][128 * kt : 128 * (kt + 1), :])
                    src = a2a_y_out[l][:].rearrange("s h b2 ll -> (s h) (b2 ll)")
                    nc.sync.dma_start(ytiles[kt][:], src[128 * kt : 128 * (kt + 1), :])
                if debug and l == 0:
                    for s in range(CORES):
                        gb = gpool.tile([64, B2 * L], F32, tag="dbgy", name="dbgy")
                        nc.sync.dma_start(gb[:], a2a_y_out[l][s].rearrange("h b2 ll -> h (b2 ll)"))
                        nc.sync.dma_start(dbg["yact0"][s].rearrange("h b2 ll -> h (b2 ll)"), gb[:])
                zps = gl.enter_context(tc.tile_pool(name=f"zps{l}", bufs=2, space="PSUM"))
                if l == 0:
                    zwp = gl.enter_context(tc.tile_pool(name=f"zw{l}", bufs=3))
                    ubp = gl.enter_context(tc.tile_pool(name=f"ub{l}", bufs=3))
                    tps = gl.enter_context(tc.tile_pool(name=f"tps{l}", bufs=2, space="PSUM"))
                    trp = gl.enter_context(tc.tile_pool(name=f"trp{l}", bufs=3))
                    u0bf = u0b_in[:].rearrange("ch b2 ll -> ch (b2 ll)")
                    for kt in range(4):
                        for ch in range(8):
                            sl = slice(ch * 512, (ch + 1) * 512)
                            psZ = zps.tile([128, 512], F32, tag="psZ", name="psZ")
                            for k2 in range(4):
                                te.matmul(psZ[:], wtiles[k2][:, kt * 128 : (kt + 1) * 128],
                                          ytiles[k2][:, sl], start=(k2 == 0), stop=(k2 == 3))
                            z1c = zwp.tile([128, 512], F32, tag="z1c", name="z1c")
                            v.tensor_scalar(z1c[:], psZ[:], brep0[:, kt : kt + 1], None, op0=OP.add)
                            psZ2 = zps.tile([128, 512], F32, tag="psZ2", name="psZ2")
                            for k2 in range(4):
                                te.matmul(psZ2[:], wtiles[k2][:, (kt + 4) * 128 : (kt + 5) * 128],
                                          ytiles[k2][:, sl], start=(k2 == 0), stop=(k2 == 3))
                            sgc = zwp.tile([128, 512], F32, tag="sgc", name="sgc")
                            sc.activation(sgc[:], psZ2[:], AF.Sigmoid, bias=brep0[:, kt + 4 : kt + 5])
                            ub = ubp.tile([128, 512], F32, tag="ub", name="ub")
                            nc.sync.dma_start(ub[:], u0bf[128 * kt : 128 * (kt + 1), sl])
                            v.tensor_mul(z1c[:], z1c[:], sgc[:])
                            v.tensor_add(z1c[:], z1c[:], ub[:])
                            # transpose the 4 l-tiles of this chunk and send
                            b2c = ch // 4
                            for c4 in range(4):
                                ccc = (ch % 4) * 4 + c4
                                psT = tps.tile([128, 128], F32, tag="psT2", name="psT2")
                                te.transpose(psT[:], z1c[:, c4 * 128 : (c4 + 1) * 128], ident[:])
                                trsb = trp.tile([128, 128], F32, tag="trsb", name="trsb")
                                sc.activation(trsb[:], psT[:], AF.Copy)
                                dst = a2a_u_in[:, :, b2c, ccc, :][2 * kt : 2 * kt + 2].rearrange(
                                    "e j hh -> j e hh")
                                nc.sync.dma_start(dst, trsb[:].rearrange("j (e hh) -> j e hh", hh=64))
                    gp.collective_compute(
                        "AllToAll", OP.bypass, replica_groups=RG,
                        ins=[a2a_u_in[:].opt()], outs=[a2a_u_out[:].opt()])
                    for s in range(CORES):
                        src = a2a_u_out[s].rearrange("j b2 c h -> j (b2 c h)")
                        dstv = u_sb[:].rearrange("j (b c h) -> j b c h", b=B, c=C)[
                            :, 2 * s : 2 * s + 2, :, :].rearrange("j b c h -> j (b c h)")
                        nc.sync.dma_start(dstv, src)
                    if debug:
                        for jj in range(2):
                            nc.sync.dma_start(
                                dbg["u1"][64 * jj : 64 * jj + 64].rearrange("j b c h -> j (b c h)"),
                                u_sb[64 * jj : 64 * jj + 64, :])
                else:
                    zw1 = gl.enter_context(tc.tile_pool(name=f"zw1{l}", bufs=3))
                    b1b = gpool.tile([128, 2 * H], F32, tag="b1b", name="b1b")
                    nc.sync.dma_start(b1b[:], b1row_in[:].broadcast_to([128, 2 * H]))
                    for b2 in range(B2):
                        for lt in range(C):
                            zz = []
                            for oh in (0, 1):
                                psW = zps.tile([128, 512], F32, tag="psW", name="psW")
                                for kt in range(4):
                                    te.matmul(psW[:],
                                              ytiles[kt][:, b2 * L + lt * T : b2 * L + (lt + 1) * T],
                                              wtiles[kt][:, oh * 512 : (oh + 1) * 512],
                                              start=(kt == 0), stop=(kt == 3))
                                zt = zw1.tile([128, 512], F32, tag=f"zt{oh}", name=f"zt{oh}")
                                v.tensor_add(zt[:], psW[:], b1b[:, oh * 512 : (oh + 1) * 512])
                                zz.append(zt)
                            sg = zw1.tile([128, 512], F32, tag="sg1", name="sg1")
                            sc.activation(sg[:], zz[1][:], AF.Sigmoid)
                            osb = zw1.tile([128, 512], F32, tag="osb", name="osb")
                            v.tensor_mul(osb[:], zz[0][:], sg[:])
                            nc.sync.dma_start(out_z[b2, lt * T : (lt + 1) * T, :], osb[:])
    nc.finalize()
    _NC_CACHE[key] = nc
    return nc


# ====================== host side ======================

def _prep_core_inputs(core, x, pars):
    hs = slice(HS * core, HS * (core + 1))
    ins = {}
    xs = x[:, :, hs]                                    # (B, L, 64)
    u0 = xs.reshape(B, C, T, HS).transpose(2, 0, 1, 3)  # (j, b, c, h)
    ins["u0"] = np.ascontiguousarray(u0)
    xb = x[B2 * core : B2 * (core + 1)]                 # (2, L, H)
    ins["u0b"] = np.ascontiguousarray(xb.transpose(2, 0, 1))
    ins["trimask"] = np.triu(np.ones((T, T), np.float32))
    ins["ident"] = np.eye(T, dtype=np.float32)

    def scan_layout(a):
        if a.ndim == 1:
            a = np.broadcast_to(a[:, None], (HS, N))
        return np.ascontiguousarray(
            a.reshape(NHB, 2, N).transpose(1, 2, 0).reshape(128, NHB))

    for l in (0, 1):
        ins[f"ldt{l}"] = scan_layout(pars[f"ldt{l}"][hs])
        ins[f"lare{l}"] = scan_layout(pars[f"lAre{l}"][hs])
        ins[f"aim{l}"] = scan_layout(pars[f"Aim{l}"][hs])
        ins[f"cre{l}"] = scan_layout(pars[f"Cre{l}"][hs])
        ins[f"cim{l}"] = scan_layout(pars[f"Cim{l}"][hs])
        ins[f"drep{l}"] = np.ascontiguousarray(
            np.broadcast_to(pars[f"D{l}"][hs][None, :], (128, HS)))
        ins[f"wt{l}"] = np.ascontiguousarray(pars[f"W{l}"].T)
    ins["brep0"] = np.ascontiguousarray(pars["b0"].reshape(8, 128).T)
    ins["b1row"] = np.ascontiguousarray(pars["b1"][None, :])
    return {k: vv.astype(np.float32) for k, vv in ins.items()}


def run(x, pars, debug=False, trace=False):
    nc = build_kernel(debug=debug)
    in_maps = [_prep_core_inputs(c, x, pars) for c in range(CORES)]
    r = run_bass_kernel_spmd(nc, in_maps, core_ids=list(range(CORES)), trace=trace)
    outs = np.stack([r.results[c]["out"] for c in range(CORES)])  # (8, 2, L, H)
    full = outs.reshape(B, L, H)
    return full, r


def kernel(**inputs):
    x = np.asarray(inputs["x"], dtype=np.float32)
    pars = {k: np.asarray(vv, dtype=np.float32) for k, vv in inputs.items() if k != "x"}
    full, _ = run(x, pars)
    return full



# revision 6
# speedup vs baseline: 1.8662x; 1.8662x over previous
"""Trainium2 Bass kernel for the 2-layer S4D block (nn_MetaS4History).

Strategy (8 cores, single launch):
  - Conv phases H-sharded (64 channels/core, full batch): chunked-SSD convolution
    with T=128 chunks: per-h matmuls (G-build, intra, injection) + a 16-step
    DVE scan for inter-chunk states.
  - GLU projections B-sharded (2 batch elems/core, full H): y-stationary
    W-moving matmuls producing [l, o] tiles directly (no output transposes).
  - Phase boundaries resharded with AllToAll collectives (3 total), bf16
    payloads.
All matmul operands in bf16 (1 cyc/row vs 4 for fp32, fast weight load);
parameter math, scan state and PSUM accumulation stay fp32.
"""
import contextlib
import math
import numpy as np
from ml_dtypes import bfloat16
import concourse.bacc as bacc
import concourse.mybir as mybir
from concourse.tile import TileContext
from concourse.bass_utils import run_bass_kernel_spmd

F32 = mybir.dt.float32
BF16 = mybir.dt.bfloat16
AF = mybir.ActivationFunctionType
OP = mybir.AluOpType

CORES = 8
B, L, H, N = 16, 2048, 512, 64
T, C = 128, 16          # chunk len, chunk count
HS = H // CORES         # 64 channels per core
B2 = B // CORES         # 2 batch per core (GLU phase)
NHB = HS // 2           # 32 h-blocks (h = 2*hblk + hpar)
LT = B2 * L // T        # 32 row-chunks in GLU phase
RG = [list(range(CORES))]

_NC_CACHE = {}


def _build_table(eng, tre, tim, seed_re, seed_im, mul_re, mul_im, wk, nhb):
    """Power table via doubling: tab[:, hb, j] = seed * mul^j, j in 0..T-1.
    tre/tim: [128, nhb*T] tiles; seed/mul: [128, nhb] APs (sliced); wk: pool."""
    t3re = tre[:].rearrange("p (h j) -> p h j", j=T)
    t3im = tim[:].rearrange("p (h j) -> p h j", j=T)
    eng.tensor_copy(t3re[:, :, 0:1], seed_re.unsqueeze(2))
    eng.tensor_copy(t3im[:, :, 0:1], seed_im.unsqueeze(2))
    mre = wk.tile([128, nhb], F32, tag="dbl_mre", name="dbl_mre")
    mim = wk.tile([128, nhb], F32, tag="dbl_mim", name="dbl_mim")
    q1 = wk.tile([128, nhb], F32, tag="dbl_q1", name="dbl_q1")
    q2 = wk.tile([128, nhb], F32, tag="dbl_q2", name="dbl_q2")
    sc1 = wk.tile([128, nhb * T // 2], F32, tag="dbl_s1", name="dbl_s1")
    eng.tensor_copy(mre[:], mul_re)
    eng.tensor_copy(mim[:], mul_im)
    m = 1
    while m < T:
        mbre = mre[:].unsqueeze(2).broadcast_to([128, nhb, m])
        mbim = mim[:].unsqueeze(2).broadcast_to([128, nhb, m])
        s1 = sc1[:].rearrange("p (h j) -> p h j", j=T // 2)[:, :, 0:m]
        src_re, src_im = t3re[:, :, 0:m], t3im[:, :, 0:m]
        dst_re, dst_im = t3re[:, :, m : 2 * m], t3im[:, :, m : 2 * m]
        eng.tensor_mul(s1, src_im, mbim)
        eng.tensor_mul(dst_re, src_re, mbre)
        eng.tensor_sub(dst_re, dst_re, s1)
        eng.tensor_mul(s1, src_im, mbre)
        eng.tensor_mul(dst_im, src_re, mbim)
        eng.tensor_add(dst_im, dst_im, s1)
        m *= 2
        if m < T:
            eng.tensor_mul(q1[:], mre[:], mre[:])
            eng.tensor_mul(q2[:], mim[:], mim[:])
            eng.tensor_mul(mim[:], mre[:], mim[:])
            eng.tensor_add(mim[:], mim[:], mim[:])
            eng.tensor_sub(mre[:], q1[:], q2[:])


def build_kernel():
    if 0 in _NC_CACHE:
        return _NC_CACHE[0]
    nc = bacc.Bacc(num_devices=CORES)
    v = nc.vector
    gp = nc.gpsimd
    sc = nc.scalar
    te = nc.tensor

    # ---------------- DRAM I/O ----------------
    u0_in = nc.dram_tensor("u0", [T, B, C, HS], BF16, kind="ExternalInput")
    u0b_in = nc.dram_tensor("u0b", [B2 * L, H], BF16, kind="ExternalInput")
    trimask_in = nc.dram_tensor("trimask", [T, T], F32, kind="ExternalInput")
    ident_in = nc.dram_tensor("ident", [T, T], F32, kind="ExternalInput")
    identb_in = nc.dram_tensor("identb", [T, T], BF16, kind="ExternalInput")
    par_in = {}
    for l in (0, 1):
        for nm in ("ldt", "lare", "aim", "cre", "cim"):
            par_in[(nm, l)] = nc.dram_tensor(f"{nm}{l}", [128, NHB], F32, kind="ExternalInput")
        par_in[("drep", l)] = nc.dram_tensor(f"drep{l}", [128, HS], F32, kind="ExternalInput")
        par_in[("wt", l)] = nc.dram_tensor(f"wt{l}", [H, 2 * H], BF16, kind="ExternalInput")
        par_in[("brow", l)] = nc.dram_tensor(f"brow{l}", [1, 2 * H], BF16, kind="ExternalInput")

    a2a_y_in = [nc.dram_tensor(f"a2aY{l}_in", [CORES, HS, B2, L], BF16) for l in (0, 1)]
    a2a_y_out = [nc.dram_tensor(f"a2aY{l}_out", [CORES, HS, B2, L], BF16) for l in (0, 1)]
    a2a_u_in = nc.dram_tensor("a2aU_in", [CORES, T, B2, C, HS], BF16)
    a2a_u_out = nc.dram_tensor("a2aU_out", [CORES, T, B2, C, HS], BF16)
    out_z = nc.dram_tensor("out", [B2, L, H], F32, kind="ExternalOutput")

    with TileContext(nc) as tc, contextlib.ExitStack() as top:
        cpool = top.enter_context(tc.tile_pool(name="consts", bufs=1))
        trimask = cpool.tile([T, T], F32, tag="trimask", name="trimask")
        ident = cpool.tile([T, T], F32, tag="ident", name="ident")
        identb = cpool.tile([T, T], BF16, tag="identb", name="identb")
        nc.sync.dma_start(trimask[:], trimask_in[:])
        nc.sync.dma_start(ident[:], ident_in[:])
        nc.sync.dma_start(identb[:], identb_in[:])
        csts = cpool.tile([128, 32], F32, tag="csts", name="csts")
        SINC = [1.0, -1.0 / 6, 1.0 / 120, -1.0 / 5040, 1.0 / 362880, -1.0 / 39916800]
        COSC = [1.0, -1.0 / 2, 1.0 / 24, -1.0 / 720, 1.0 / 40320, -1.0 / 3628800]
        for k in range(6):
            nc.any.memset(csts[:, k : k + 1], SINC[k])
            nc.any.memset(csts[:, 6 + k : 7 + k], COSC[k])
        nc.any.memset(csts[:, 12:13], -1.0)
        nc.any.memset(csts[:, 13:14], 2.0)
        nc.any.memset(csts[:, 14:15], 1.0 / 16)
        for k in range(11):
            nc.any.memset(csts[:, 16 + k : 17 + k], 1.0 / math.factorial(k))
        nc.any.memset(csts[:, 27:28], 1.0 / 8)
        onesb = cpool.tile([1, T], BF16, tag="onesb", name="onesb")
        nc.any.memset(onesb[:], 1.0)

        upool = top.enter_context(tc.tile_pool(name="u", bufs=1))
        u_sb = upool.tile([T, B * C * HS], BF16, tag="u_sb", name="u_sb")  # [j,(b,c,h)]
        nc.sync.dma_start(u_sb[:], u0_in[:].rearrange("j b c h -> j (b c h)"))

        def u_slice(h, bq=None):
            b0, nb = (0, B) if bq is None else (bq * 8, 8)
            ap = u_sb[:].rearrange("j (b c h) -> j b c h", b=B, c=C)
            return ap[:, b0 : b0 + nb, :, h]

        for l in (0, 1):
            # ======== CONV PHASE (H-shard) ========
            with contextlib.ExitStack() as cv:
                pp = cv.enter_context(tc.tile_pool(name=f"par{l}", bufs=1))
                P = {}
                for nm in ("ldt", "lare", "aim", "cre", "cim"):
                    P[nm] = pp.tile([128, NHB], F32, tag=f"p_{nm}", name=f"p_{nm}")
                    nc.sync.dma_start(P[nm][:], par_in[(nm, l)][:])
                drep = pp.tile([128, HS], F32, tag="p_drep", name="p_drep")
                nc.sync.dma_start(drep[:], par_in[("drep", l)][:])

                def wk(tag):
                    return pp.tile([128, NHB], F32, tag=tag, name=tag)[:]

                neg1 = csts[:, 12:13]
                two = csts[:, 13:14]
                s16 = csts[:, 14:15]

                def exp_poly(out, x):
                    """out = e^x via (T10(x/8))^8; |x| <= 8. Accurate to ~1e-7."""
                    ea = wk("exp_a")
                    et = wk("exp_t")
                    v.tensor_scalar(ea, x, csts[:, 27:28], None, op0=OP.mult)  # y = x/8
                    v.tensor_scalar(et, ea, csts[:, 26:27], csts[:, 25:26], op0=OP.mult, op1=OP.add)
                    for k in range(8, -1, -1):
                        v.tensor_mul(et, et, ea)
                        v.tensor_scalar(et, et, csts[:, 16 + k : 17 + k], None, op0=OP.add)
                    for _ in range(3):
                        v.tensor_mul(et, et, et)
                    v.tensor_copy(out, et)

                dt, eA = wk("dt"), wk("eA")
                exp_poly(dt, P["ldt"][:])
                exp_poly(eA, P["lare"][:])
                dtAre, dtAim = wk("dtAre"), wk("dtAim")
                v.scalar_tensor_tensor(dtAre, dt, -1.0, eA, op0=OP.mult, op1=OP.mult)
                v.tensor_mul(dtAim, dt, P["aim"][:])
                mag = wk("mag")
                exp_poly(mag, dtAre)
                q, x2 = wk("q"), wk("x2")
                v.tensor_scalar(q, dtAim, s16, None, op0=OP.mult)
                v.tensor_mul(x2, q, q)
                acc, t1, t2 = wk("acc"), wk("t1"), wk("t2")
                cr, ci = wk("cr"), wk("ci")
                v.tensor_scalar(acc, x2, csts[:, 5:6], csts[:, 4:5], op0=OP.mult, op1=OP.add)
                for k in (3, 2, 1, 0):
                    v.tensor_mul(t1, acc, x2)
                    v.tensor_scalar(acc, t1, csts[:, k : k + 1], None, op0=OP.add)
                v.tensor_mul(ci, acc, q)
                v.tensor_scalar(acc, x2, csts[:, 11:12], csts[:, 10:11], op0=OP.mult, op1=OP.add)
                for k in (9, 8, 7, 6):
                    v.tensor_mul(t1, acc, x2)
                    v.tensor_scalar(acc, t1, csts[:, k : k + 1], None, op0=OP.add)
                v.tensor_copy(cr, acc)
                for _ in range(4):
                    v.tensor_mul(t1, cr, cr)
                    v.tensor_mul(t2, ci, ci)
                    v.scalar_tensor_tensor(acc, cr, 2.0, ci, op0=OP.mult, op1=OP.mult)
                    v.tensor_sub(cr, t1, t2)
                    v.tensor_copy(ci, acc)
                wre, wim = wk("wre"), wk("wim")
                v.tensor_mul(wre, mag, cr)
                v.tensor_mul(wim, mag, ci)
                m2, im2 = wk("m2"), wk("im2")
                v.tensor_mul(m2, mag, mag)
                v.reciprocal(im2, m2)
                rpre, rpim = wk("rpre"), wk("rpim")
                v.tensor_mul(rpre, wre, im2)
                v.tensor_mul(rpim, wim, im2)
                wm1re = wk("wm1re")
                v.tensor_scalar(wm1re, wre, neg1, None, op0=OP.add)
                tre, tim = wk("tre"), wk("tim")
                v.tensor_mul(t1, P["cre"][:], wm1re)
                v.tensor_mul(t2, P["cim"][:], wim)
                v.tensor_sub(tre, t1, t2)
                v.tensor_mul(t1, P["cre"][:], wim)
                v.tensor_mul(t2, P["cim"][:], wm1re)
                v.tensor_add(tim, t1, t2)
                den, invd = wk("den"), wk("invd")
                v.tensor_mul(t1, eA, eA)
                v.tensor_mul(t2, P["aim"][:], P["aim"][:])
                v.tensor_add(den, t1, t2)
                v.reciprocal(invd, den)
                ccr, cci = wk("ccr"), wk("cci")
                v.tensor_mul(t1, tre, eA)
                v.tensor_mul(t2, tim, P["aim"][:])
                v.tensor_sub(acc, t2, t1)
                v.tensor_mul(ccr, acc, invd)
                v.tensor_mul(t1, tre, P["aim"][:])
                v.tensor_mul(t2, tim, eA)
                v.tensor_add(acc, t1, t2)
                v.tensor_mul(t1, acc, invd)
                v.tensor_scalar(cci, t1, neg1, None, op0=OP.mult)
                esr, esi = wk("esr"), wk("esi")
                v.tensor_mul(t1, ccr, wre)
                v.tensor_mul(t2, cci, wim)
                v.tensor_sub(acc, t1, t2)
                v.tensor_scalar(esr, acc, two, None, op0=OP.mult)
                v.tensor_mul(t1, ccr, wim)
                v.tensor_mul(t2, cci, wre)
                v.tensor_add(acc, t1, t2)
                v.tensor_scalar(esi, acc, two, None, op0=OP.mult)
                wtr, wti = wk("wtr"), wk("wti")
                v.tensor_copy(wtr, wre)
                v.tensor_copy(wti, wim)
                for _ in range(7):
                    v.tensor_mul(t1, wtr, wtr)
                    v.tensor_mul(t2, wti, wti)
                    v.scalar_tensor_tensor(acc, wtr, 2.0, wti, op0=OP.mult, op1=OP.mult)
                    v.tensor_sub(wtr, t1, t2)
                    v.tensor_copy(wti, acc)
                dre, dim_ = wk("dre"), wk("dim")
                v.tensor_copy(dre, wtr)
                v.tensor_scalar(dim_, wti, neg1, None, op0=OP.mult)

                # ---------- bf16 power tables (built fp32, rounded once) ----------
                tabp = cv.enter_context(tc.tile_pool(name=f"tab{l}", bufs=1))
                Rb_re = tabp.tile([128, NHB * T], BF16, tag="Rb_re", name="Rb_re")
                Rb_im = tabp.tile([128, NHB * T], BF16, tag="Rb_im", name="Rb_im")
                Eb_re = tabp.tile([128, NHB * T], BF16, tag="Eb_re", name="Eb_re")
                Eb_im = tabp.tile([128, NHB * T], BF16, tag="Eb_im", name="Eb_im")
                with tc.tile_pool(name=f"tf{l}", bufs=1) as ftp:
                    Rf_re = ftp.tile([128, NHB * T], F32, tag="Rf_re", name="Rf_re")
                    Rf_im = ftp.tile([128, NHB * T], F32, tag="Rf_im", name="Rf_im")
                    Ef_re = ftp.tile([128, NHB * T], F32, tag="Ef_re", name="Ef_re")
                    Ef_im = ftp.tile([128, NHB * T], F32, tag="Ef_im", name="Ef_im")
                    with tc.tile_pool(name=f"dblR{l}", bufs=1) as dwkR, \
                         tc.tile_pool(name=f"dblE{l}", bufs=1) as dwkE:
                        _build_table(gp, Rf_re, Rf_im, rpre, rpim, rpre, rpim, dwkR, NHB)
                        _build_table(v, Ef_re, Ef_im, esr, esi, wre, wim, dwkE, NHB)
                    sc.activation(Rb_re[:], Rf_re[:], AF.Copy)
                    sc.activation(Rb_im[:], Rf_im[:], AF.Copy)
                    sc.activation(Eb_re[:], Ef_re[:], AF.Copy)
                    sc.activation(Eb_im[:], Ef_im[:], AF.Copy)

                # ---------- collection: X[n, (b,c)] = R' @ u ----------
                stp = cv.enter_context(tc.tile_pool(name=f"st{l}", bufs=1))
                X_re = stp.tile([128, NHB * B * C], BF16, tag="X_re", name="X_re")
                X_im = stp.tile([128, NHB * B * C], BF16, tag="X_im", name="X_im")
                X_re4 = X_re[:].rearrange("p (h b c) -> p h b c", b=B, c=C)
                X_im4 = X_im[:].rearrange("p (h b c) -> p h b c", b=B, c=C)

                with tc.tile_pool(name=f"wsl{l}", bufs=3) as wslp, \
                     tc.tile_pool(name=f"pst{l}", bufs=2, space="PSUM") as pstp, \
                     tc.tile_pool(name=f"psc{l}", bufs=2, space="PSUM") as pscp:
                    for k in range(NHB):
                        wsl = [wslp.tile([128, T], BF16, tag=f"wsl{comp}", name=f"wsl{comp}")
                               for comp in (0, 1)]
                        for comp, Rt in enumerate((Rb_re, Rb_im)):
                            psT = pstp.tile([128, T], BF16, tag="psT", name="psT")
                            te.transpose(psT[:], Rt[:, k * T : (k + 1) * T], identb[:])
                            sc.activation(wsl[comp][:], psT[:], AF.Copy)
                        psr = pscp.tile([128, B * C], F32, tag="psr", name="psr")
                        psi = pscp.tile([128, B * C], F32, tag="psi", name="psi")
                        for hp in (0, 1):
                            h = 2 * k + hp
                            us = u_slice(h)
                            te.matmul(psr[64 * hp : 64 * hp + 64, :],
                                      wsl[0][:, 64 * hp : 64 * hp + 64], us, start=True, stop=True)
                            te.matmul(psi[64 * hp : 64 * hp + 64, :],
                                      wsl[1][:, 64 * hp : 64 * hp + 64], us, start=True, stop=True)
                        sc.activation(X_re4[:, k, :, :], psr[:], AF.Copy)
                        sc.activation(X_im4[:, k, :, :], psi[:], AF.Copy)

                # ---------- scan (in place: X becomes Sacc; states fp32) ----------
                with tc.tile_pool(name=f"scan{l}", bufs=1) as sp:
                    def stile(nm):
                        return sp.tile([128, NHB * B], F32, tag=nm, name=nm)[:].rearrange(
                            "p (h b) -> p h b", b=B)
                    Sr3, Si3 = stile("Sr"), stile("Si")
                    t_r3, t_i3 = stile("tm_r"), stile("tm_i")
                    w13, w23 = stile("w1"), stile("w2")
                    nc.any.memset(Sr3, 0.0)
                    nc.any.memset(Si3, 0.0)
                    dreb = dre.unsqueeze(2).broadcast_to([128, NHB, B])
                    dimb = dim_.unsqueeze(2).broadcast_to([128, NHB, B])
                    for ccc in range(C):
                        xr, xi = X_re4[:, :, :, ccc], X_im4[:, :, :, ccc]
                        v.tensor_add(t_r3, Sr3, xr)
                        gp.tensor_add(t_i3, Si3, xi)
                        sc.activation(xr, Sr3, AF.Copy)
                        sc.activation(xi, Si3, AF.Copy)
                        v.tensor_mul(w13, t_r3, dreb)
                        v.tensor_mul(w23, t_i3, dimb)
                        v.tensor_sub(Sr3, w13, w23)
                        v.tensor_mul(w13, t_i3, dreb)
                        v.tensor_mul(w23, t_r3, dimb)
                        v.tensor_add(Si3, w13, w23)

                # ---------- per-h conv ----------
                gt_pool = cv.enter_context(tc.tile_pool(name=f"gt{l}", bufs=3))
                ya_pool = cv.enter_context(tc.tile_pool(name=f"ya{l}", bufs=2))
                gp_ps = cv.enter_context(tc.tile_pool(name=f"gps{l}", bufs=2, space="PSUM"))
                cv_ps = cv.enter_context(tc.tile_pool(name=f"cvps{l}", bufs=2, space="PSUM"))
                HG = 8
                yg = [None, None]
                for h in range(HS):
                    hp, hb = h & 1, h >> 1
                    base = 64 * hp
                    er = Eb_re[base : base + 64, hb * T : (hb + 1) * T]
                    ei = Eb_im[base : base + 64, hb * T : (hb + 1) * T]
                    rr = Rb_re[base : base + 64, hb * T : (hb + 1) * T]
                    ri = Rb_im[base : base + 64, hb * T : (hb + 1) * T]
                    psG = gp_ps.tile([128, T], F32, tag="psG", name="psG")
                    te.matmul(psG[:], rr, er, start=True, stop=False)
                    te.matmul(psG[:], ri, ei, start=False, stop=True)
                    GTt = gt_pool.tile([128, T], BF16, tag="GTt", name="GTt")
                    GT = gt_pool.tile([128, T], BF16, tag="GT", name="GT")
                    v.tensor_mul(GTt[:], psG[:], trimask[:])
                    v.scalar_tensor_tensor(GT[:], ident[:], drep[:, h : h + 1], GTt[:],
                                           op0=OP.mult, op1=OP.add)
                    if h % HG == 0:
                        yg = [ya_pool.tile([128, HG * T], BF16, tag=f"yg{qq}", name=f"yg{qq}")
                              for qq in (0, 1)]
                    for qq in (0, 1):
                        ps = cv_ps.tile([128, T], F32, tag="ps", name="ps")
                        lu = u_slice(h, qq)
                        te.matmul(ps[:], lu, GT[:], start=True, stop=False)
                        lr = X_re[base : base + 64,
                                  hb * B * C + qq * 128 : hb * B * C + qq * 128 + 128]
                        li = X_im[base : base + 64,
                                  hb * B * C + qq * 128 : hb * B * C + qq * 128 + 128]
                        te.matmul(ps[:], lr, er, start=False, stop=False)
                        te.matmul(ps[:], li, ei, start=False, stop=True)
                        sc.activation(yg[qq][:, (h % HG) * T : (h % HG + 1) * T], ps[:],
                                      AF.Gelu_apprx_tanh)
                    if h % HG == HG - 1:
                        hg0 = h - HG + 1
                        for qq in (0, 1):
                            ygv = yg[qq][:].rearrange("bc (hh2 j) -> bc hh2 j", j=T)
                            for dd in range(4):
                                d = qq * 4 + dd
                                dst = a2a_y_in[l][d, hg0 : hg0 + HG, :, :].rearrange(
                                    "hh2 b2 (c j) -> (b2 c) hh2 j", j=T)
                                nc.sync.dma_start(dst, ygv[32 * dd : 32 * dd + 32, :, :])

            # ======== AllToAll y ========
            gp.collective_compute(
                "AllToAll", OP.bypass, replica_groups=RG,
                ins=[a2a_y_in[l][:].opt()], outs=[a2a_y_out[l][:].opt()])

            # ======== GLU PHASE (B-shard): z[l, o] = y^T W, GLU, reshard ========
            with contextlib.ExitStack() as gl:
                gpool = gl.enter_context(tc.tile_pool(name=f"glu{l}", bufs=1))
                wtiles = [gpool.tile([128, 2 * H], BF16, tag=f"wt{k}", name=f"wt{k}") for k in range(4)]
                ytiles = [gpool.tile([128, B2 * L], BF16, tag=f"yk{k}", name=f"yk{k}") for k in range(4)]
                for kt in range(4):
                    nc.sync.dma_start(wtiles[kt][:], par_in[("wt", l)][128 * kt : 128 * (kt + 1), :])
                    src = a2a_y_out[l][:].rearrange("s h b2 ll -> (s h) (b2 ll)")
                    nc.sync.dma_start(ytiles[kt][:], src[128 * kt : 128 * (kt + 1), :])
                brow = gpool.tile([1, 2 * H], BF16, tag="brow", name="brow")
                nc.sync.dma_start(brow[:], par_in[("brow", l)][:])
                zps = gl.enter_context(tc.tile_pool(name=f"zps{l}", bufs=2, space="PSUM"))
                zwp = gl.enter_context(tc.tile_pool(name=f"zw{l}", bufs=3))
                ubp = gl.enter_context(tc.tile_pool(name=f"ub{l}", bufs=3))
                zdt = BF16 if l == 0 else F32
                for lt in range(LT):
                    sl = slice(lt * T, (lt + 1) * T)
                    ps1 = zps.tile([128, H], F32, tag="ps1", name="ps1")
                    ps2 = zps.tile([128, H], F32, tag="ps2", name="ps2")
                    te.matmul(ps1[:], onesb[:], brow[:, 0:H], start=True, stop=False)
                    te.matmul(ps2[:], onesb[:], brow[:, H : 2 * H], start=True, stop=False)
                    for kt in range(4):
                        te.matmul(ps1[:], ytiles[kt][:, sl], wtiles[kt][:, 0:H],
                                  start=False, stop=(kt == 3))
                        te.matmul(ps2[:], ytiles[kt][:, sl], wtiles[kt][:, H : 2 * H],
                                  start=False, stop=(kt == 3))
                    z1 = zwp.tile([128, H], zdt, tag="z1", name="z1")
                    sg = zwp.tile([128, H], zdt, tag="sg", name="sg")
                    sc.activation(sg[:], ps2[:], AF.Sigmoid)
                    v.tensor_mul(z1[:], ps1[:], sg[:])
                    b2c, ccc = lt // C, lt % C
                    if l == 0:
                        ub = ubp.tile([128, H], BF16, tag="ub", name="ub")
                        nc.sync.dma_start(ub[:], u0b_in[lt * T : (lt + 1) * T, :])
                        gp.tensor_add(z1[:], z1[:], ub[:])
                        for d in range(CORES):
                            nc.sync.dma_start(a2a_u_in[d, :, b2c, ccc, :],
                                              z1[:, HS * d : HS * (d + 1)])
                    else:
                        nc.sync.dma_start(out_z[b2c, ccc * T : (ccc + 1) * T, :], z1[:])
                if l == 0:
                    gp.collective_compute(
                        "AllToAll", OP.bypass, replica_groups=RG,
                        ins=[a2a_u_in[:].opt()], outs=[a2a_u_out[:].opt()])
                    for s in range(CORES):
                        src = a2a_u_out[s].rearrange("j b2 c h -> j (b2 c h)")
                        dstv = u_sb[:].rearrange("j (b c h) -> j b c h", b=B, c=C)[
                            :, 2 * s : 2 * s + 2, :, :].rearrange("j b c h -> j (b c h)")
                        nc.sync.dma_start(dstv, src)
    nc.finalize()
    _NC_CACHE[0] = nc
    return nc


# ====================== host side ======================

def _prep_core_inputs(core, x, pars):
    hs = slice(HS * core, HS * (core + 1))
    ins = {}
    xs = x[:, :, hs]                                    # (B, L, 64)
    u0 = xs.reshape(B, C, T, HS).transpose(2, 0, 1, 3)  # (j, b, c, h)
    ins["u0"] = np.ascontiguousarray(u0).astype(bfloat16)
    xb = x[B2 * core : B2 * (core + 1)]                 # (2, L, H)
    ins["u0b"] = xb.reshape(B2 * L, H).astype(bfloat16)
    ins["trimask"] = np.triu(np.ones((T, T), np.float32))
    ins["ident"] = np.eye(T, dtype=np.float32)
    ins["identb"] = np.eye(T, dtype=np.float32).astype(bfloat16)

    def scan_layout(a):
        if a.ndim == 1:
            a = np.broadcast_to(a[:, None], (HS, N))
        return np.ascontiguousarray(
            a.reshape(NHB, 2, N).transpose(1, 2, 0).reshape(128, NHB)).astype(np.float32)

    for l in (0, 1):
        ins[f"ldt{l}"] = scan_layout(pars[f"ldt{l}"][hs])
        ins[f"lare{l}"] = scan_layout(pars[f"lAre{l}"][hs])
        ins[f"aim{l}"] = scan_layout(pars[f"Aim{l}"][hs])
        ins[f"cre{l}"] = scan_layout(pars[f"Cre{l}"][hs])
        ins[f"cim{l}"] = scan_layout(pars[f"Cim{l}"][hs])
        ins[f"drep{l}"] = np.ascontiguousarray(
            np.broadcast_to(pars[f"D{l}"][hs][None, :], (128, HS))).astype(np.float32)
        ins[f"wt{l}"] = np.ascontiguousarray(pars[f"W{l}"].T).astype(bfloat16)
        ins[f"brow{l}"] = pars[f"b{l}"][None, :].astype(bfloat16)
    return ins


def run(x, pars, debug=False, trace=False):
    nc = build_kernel()
    in_maps = [_prep_core_inputs(c, x, pars) for c in range(CORES)]
    r = run_bass_kernel_spmd(nc, in_maps, core_ids=list(range(CORES)), trace=trace)
    outs = np.stack([r.results[c]["out"] for c in range(CORES)])  # (8, 2, L, H)
    full = outs.reshape(B, L, H)
    return full, r


def kernel(**inputs):
    x = np.asarray(inputs["x"], dtype=np.float32)
    pars = {k: np.asarray(vv, dtype=np.float32) for k, vv in inputs.items() if k != "x"}
    full, _ = run(x, pars)
    return full


# revision 7
# speedup vs baseline: 1.9935x; 1.0682x over previous
"""Trainium2 Bass kernel for the 2-layer S4D block (nn_MetaS4History).

Strategy (8 cores, single launch):
  - Conv phases H-sharded (64 channels/core, full batch): chunked-SSD convolution
    with T=128 chunks: per-h matmuls (G-build, intra, injection) + a 16-step
    scan for inter-chunk states, split across Vector/GpSimd with ping-pong
    state buffers.
  - GLU projections B-sharded (2 batch elems/core, full H): y-stationary
    W-moving matmuls producing [l, o] tiles directly (no output transposes).
  - Phase boundaries resharded with AllToAll collectives (3 total), bf16
    payloads. Layer-1 parameter math + power tables are computed during the
    layer-0 collective/GLU window so they stay off the critical path.
All matmul operands in bf16 (1 cyc/row vs 4 for fp32, fast weight load);
parameter math, scan state and PSUM accumulation stay fp32.
"""
import contextlib
import math
import numpy as np
from ml_dtypes import bfloat16
import concourse.bacc as bacc
import concourse.mybir as mybir
from concourse.tile import TileContext
from concourse.bass_utils import run_bass_kernel_spmd

F32 = mybir.dt.float32
BF16 = mybir.dt.bfloat16
AF = mybir.ActivationFunctionType
OP = mybir.AluOpType

CORES = 8
B, L, H, N = 16, 2048, 512, 64
T, C = 128, 16          # chunk len, chunk count
HS = H // CORES         # 64 channels per core
B2 = B // CORES         # 2 batch per core (GLU phase)
NHB = HS // 2           # 32 h-blocks (h = 2*hblk + hpar)
LT = B2 * L // T        # 32 row-chunks in GLU phase
HV = 20                 # h-blocks scanned on Vector (rest on GpSimd)
RG = [list(range(CORES))]

_NC_CACHE = {}


def _build_table(eng, tre, tim, seed_re, seed_im, mul_re, mul_im, wk, nhb):
    """Power table via doubling: tab[:, hb, j] = seed * mul^j, j in 0..T-1.
    tre/tim: [128, nhb*T] tiles; seed/mul: [128, nhb] APs (sliced); wk: pool."""
    t3re = tre[:].rearrange("p (h j) -> p h j", j=T)
    t3im = tim[:].rearrange("p (h j) -> p h j", j=T)
    eng.tensor_copy(t3re[:, :, 0:1], seed_re.unsqueeze(2))
    eng.tensor_copy(t3im[:, :, 0:1], seed_im.unsqueeze(2))
    mre = wk.tile([128, nhb], F32, tag="dbl_mre", name="dbl_mre")
    mim = wk.tile([128, nhb], F32, tag="dbl_mim", name="dbl_mim")
    q1 = wk.tile([128, nhb], F32, tag="dbl_q1", name="dbl_q1")
    q2 = wk.tile([128, nhb], F32, tag="dbl_q2", name="dbl_q2")
    sc1 = wk.tile([128, nhb * T // 2], F32, tag="dbl_s1", name="dbl_s1")
    eng.tensor_copy(mre[:], mul_re)
    eng.tensor_copy(mim[:], mul_im)
    m = 1
    while m < T:
        mbre = mre[:].unsqueeze(2).broadcast_to([128, nhb, m])
        mbim = mim[:].unsqueeze(2).broadcast_to([128, nhb, m])
        s1 = sc1[:].rearrange("p (h j) -> p h j", j=T // 2)[:, :, 0:m]
        src_re, src_im = t3re[:, :, 0:m], t3im[:, :, 0:m]
        dst_re, dst_im = t3re[:, :, m : 2 * m], t3im[:, :, m : 2 * m]
        eng.tensor_mul(s1, src_im, mbim)
        eng.tensor_mul(dst_re, src_re, mbre)
        eng.tensor_sub(dst_re, dst_re, s1)
        eng.tensor_mul(s1, src_im, mbre)
        eng.tensor_mul(dst_im, src_re, mbim)
        eng.tensor_add(dst_im, dst_im, s1)
        m *= 2
        if m < T:
            eng.tensor_mul(q1[:], mre[:], mre[:])
            eng.tensor_mul(q2[:], mim[:], mim[:])
            eng.tensor_mul(mim[:], mre[:], mim[:])
            eng.tensor_add(mim[:], mim[:], mim[:])
            eng.tensor_sub(mre[:], q1[:], q2[:])


def build_kernel(bias_zero=True):
    key = bias_zero
    if key in _NC_CACHE:
        return _NC_CACHE[key]
    nc = bacc.Bacc(num_devices=CORES)
    v = nc.vector
    gp = nc.gpsimd
    sc = nc.scalar
    te = nc.tensor

    # ---------------- DRAM I/O ----------------
    u0_in = nc.dram_tensor("u0", [T, B, C, HS], BF16, kind="ExternalInput")
    u0b_in = nc.dram_tensor("u0b", [B2 * L, H], BF16, kind="ExternalInput")
    trimask_in = nc.dram_tensor("trimask", [T, T], F32, kind="ExternalInput")
    ident_in = nc.dram_tensor("ident", [T, T], F32, kind="ExternalInput")
    identb_in = nc.dram_tensor("identb", [T, T], BF16, kind="ExternalInput")
    par_in = {}
    for l in (0, 1):
        for nm in ("ldt", "lare", "aim", "cre", "cim"):
            par_in[(nm, l)] = nc.dram_tensor(f"{nm}{l}", [128, NHB], F32, kind="ExternalInput")
        par_in[("drep", l)] = nc.dram_tensor(f"drep{l}", [128, HS], F32, kind="ExternalInput")
        par_in[("wt", l)] = nc.dram_tensor(f"wt{l}", [H, 2 * H], BF16, kind="ExternalInput")
        par_in[("brow", l)] = nc.dram_tensor(f"brow{l}", [1, 2 * H], BF16, kind="ExternalInput")

    a2a_y_in = [nc.dram_tensor(f"a2aY{l}_in", [CORES, HS, B2, L], BF16) for l in (0, 1)]
    a2a_y_out = [nc.dram_tensor(f"a2aY{l}_out", [CORES, HS, B2, L], BF16) for l in (0, 1)]
    a2a_u_in = nc.dram_tensor("a2aU_in", [CORES, T, B2, C, HS], BF16)
    a2a_u_out = nc.dram_tensor("a2aU_out", [CORES, T, B2, C, HS], BF16)
    out_z = nc.dram_tensor("out", [B2, L, H], F32, kind="ExternalOutput")

    with TileContext(nc) as tc, contextlib.ExitStack() as top:
        cpool = top.enter_context(tc.tile_pool(name="consts", bufs=1))
        trimask = cpool.tile([T, T], F32, tag="trimask", name="trimask")
        ident = cpool.tile([T, T], F32, tag="ident", name="ident")
        identb = cpool.tile([T, T], BF16, tag="identb", name="identb")
        nc.sync.dma_start(trimask[:], trimask_in[:])
        nc.sync.dma_start(ident[:], ident_in[:])
        nc.sync.dma_start(identb[:], identb_in[:])
        csts = cpool.tile([128, 32], F32, tag="csts", name="csts")
        SINC = [1.0, -1.0 / 6, 1.0 / 120, -1.0 / 5040, 1.0 / 362880, -1.0 / 39916800]
        COSC = [1.0, -1.0 / 2, 1.0 / 24, -1.0 / 720, 1.0 / 40320, -1.0 / 3628800]
        for k in range(6):
            nc.any.memset(csts[:, k : k + 1], SINC[k])
            nc.any.memset(csts[:, 6 + k : 7 + k], COSC[k])
        nc.any.memset(csts[:, 12:13], -1.0)
        nc.any.memset(csts[:, 13:14], 2.0)
        nc.any.memset(csts[:, 14:15], 1.0 / 16)
        for k in range(11):
            nc.any.memset(csts[:, 16 + k : 17 + k], 1.0 / math.factorial(k))
        nc.any.memset(csts[:, 27:28], 1.0 / 8)
        onesb = cpool.tile([1, T], BF16, tag="onesb", name="onesb")
        nc.any.memset(onesb[:], 1.0)

        upool = top.enter_context(tc.tile_pool(name="u", bufs=1))
        u_sb = upool.tile([T, B * C * HS], BF16, tag="u_sb", name="u_sb")  # [j,(b,c,h)]
        nc.sync.dma_start(u_sb[:], u0_in[:].rearrange("j b c h -> j (b c h)"))

        def u_slice(h, bq=None):
            b0, nb = (0, B) if bq is None else (bq * 8, 8)
            ap = u_sb[:].rearrange("j (b c h) -> j b c h", b=B, c=C)
            return ap[:, b0 : b0 + nb, :, h]

        neg1 = csts[:, 12:13]
        two = csts[:, 13:14]
        s16 = csts[:, 14:15]

        # ---------- per-layer constants: params, D-rep, bf16 power tables ----
        def make_layer_consts(l, stack):
            lcp = stack.enter_context(tc.tile_pool(name=f"lc{l}", bufs=1))
            LC = {"drep": lcp.tile([128, HS], F32, tag="p_drep", name="p_drep")}
            nc.sync.dma_start(LC["drep"][:], par_in[("drep", l)][:])
            for nm in ("Rb_re", "Rb_im", "Eb_re", "Eb_im"):
                LC[nm] = lcp.tile([128, NHB * T], BF16, tag=nm, name=nm)
            LC["dre"] = lcp.tile([128, NHB], F32, tag="dreS", name="dreS")
            LC["dim"] = lcp.tile([128, NHB], F32, tag="dimS", name="dimS")

            with tc.tile_pool(name=f"par{l}", bufs=1) as pp:
                P = {}
                for nm in ("ldt", "lare", "aim", "cre", "cim"):
                    P[nm] = pp.tile([128, NHB], F32, tag=f"p_{nm}", name=f"p_{nm}")
                    nc.sync.dma_start(P[nm][:], par_in[(nm, l)][:])

                def wk(tag):
                    return pp.tile([128, NHB], F32, tag=tag, name=tag)[:]

                def exp_poly(out, x):
                    """out = e^x via (T10(x/8))^8; |x| <= 8. Accurate to ~1e-7."""
                    ea = wk("exp_a")
                    et = wk("exp_t")
                    v.tensor_scalar(ea, x, csts[:, 27:28], None, op0=OP.mult)  # y = x/8
                    v.tensor_scalar(et, ea, csts[:, 26:27], csts[:, 25:26], op0=OP.mult, op1=OP.add)
                    for k in range(8, -1, -1):
                        v.tensor_mul(et, et, ea)
                        v.tensor_scalar(et, et, csts[:, 16 + k : 17 + k], None, op0=OP.add)
                    for _ in range(3):
                        v.tensor_mul(et, et, et)
                    v.tensor_copy(out, et)

                dt, eA = wk("dt"), wk("eA")
                exp_poly(dt, P["ldt"][:])
                exp_poly(eA, P["lare"][:])
                dtAre, dtAim = wk("dtAre"), wk("dtAim")
                v.scalar_tensor_tensor(dtAre, dt, -1.0, eA, op0=OP.mult, op1=OP.mult)
                v.tensor_mul(dtAim, dt, P["aim"][:])
                mag = wk("mag")
                exp_poly(mag, dtAre)
                q, x2 = wk("q"), wk("x2")
                v.tensor_scalar(q, dtAim, s16, None, op0=OP.mult)
                v.tensor_mul(x2, q, q)
                acc, t1, t2 = wk("acc"), wk("t1"), wk("t2")
                cr, ci = wk("cr"), wk("ci")
                v.tensor_scalar(acc, x2, csts[:, 5:6], csts[:, 4:5], op0=OP.mult, op1=OP.add)
                for k in (3, 2, 1, 0):
                    v.tensor_mul(t1, acc, x2)
                    v.tensor_scalar(acc, t1, csts[:, k : k + 1], None, op0=OP.add)
                v.tensor_mul(ci, acc, q)
                v.tensor_scalar(acc, x2, csts[:, 11:12], csts[:, 10:11], op0=OP.mult, op1=OP.add)
                for k in (9, 8, 7, 6):
                    v.tensor_mul(t1, acc, x2)
                    v.tensor_scalar(acc, t1, csts[:, k : k + 1], None, op0=OP.add)
                v.tensor_copy(cr, acc)
                for _ in range(4):
                    v.tensor_mul(t1, cr, cr)
                    v.tensor_mul(t2, ci, ci)
                    v.scalar_tensor_tensor(acc, cr, 2.0, ci, op0=OP.mult, op1=OP.mult)
                    v.tensor_sub(cr, t1, t2)
                    v.tensor_copy(ci, acc)
                wre, wim = wk("wre"), wk("wim")
                v.tensor_mul(wre, mag, cr)
                v.tensor_mul(wim, mag, ci)
                m2, im2 = wk("m2"), wk("im2")
                v.tensor_mul(m2, mag, mag)
                v.reciprocal(im2, m2)
                rpre, rpim = wk("rpre"), wk("rpim")
                v.tensor_mul(rpre, wre, im2)
                v.tensor_mul(rpim, wim, im2)
                wm1re = wk("wm1re")
                v.tensor_scalar(wm1re, wre, neg1, None, op0=OP.add)
                tre, tim = wk("tre"), wk("tim")
                v.tensor_mul(t1, P["cre"][:], wm1re)
                v.tensor_mul(t2, P["cim"][:], wim)
                v.tensor_sub(tre, t1, t2)
                v.tensor_mul(t1, P["cre"][:], wim)
                v.tensor_mul(t2, P["cim"][:], wm1re)
                v.tensor_add(tim, t1, t2)
                den, invd = wk("den"), wk("invd")
                v.tensor_mul(t1, eA, eA)
                v.tensor_mul(t2, P["aim"][:], P["aim"][:])
                v.tensor_add(den, t1, t2)
                v.reciprocal(invd, den)
                ccr, cci = wk("ccr"), wk("cci")
                v.tensor_mul(t1, tre, eA)
                v.tensor_mul(t2, tim, P["aim"][:])
                v.tensor_sub(acc, t2, t1)
                v.tensor_mul(ccr, acc, invd)
                v.tensor_mul(t1, tre, P["aim"][:])
                v.tensor_mul(t2, tim, eA)
                v.tensor_add(acc, t1, t2)
                v.tensor_mul(t1, acc, invd)
                v.tensor_scalar(cci, t1, neg1, None, op0=OP.mult)
                esr, esi = wk("esr"), wk("esi")
                v.tensor_mul(t1, ccr, wre)
                v.tensor_mul(t2, cci, wim)
                v.tensor_sub(acc, t1, t2)
                v.tensor_scalar(esr, acc, two, None, op0=OP.mult)
                v.tensor_mul(t1, ccr, wim)
                v.tensor_mul(t2, cci, wre)
                v.tensor_add(acc, t1, t2)
                v.tensor_scalar(esi, acc, two, None, op0=OP.mult)
                wtr, wti = wk("wtr"), wk("wti")
                v.tensor_copy(wtr, wre)
                v.tensor_copy(wti, wim)
                for _ in range(7):
                    v.tensor_mul(t1, wtr, wtr)
                    v.tensor_mul(t2, wti, wti)
                    v.scalar_tensor_tensor(acc, wtr, 2.0, wti, op0=OP.mult, op1=OP.mult)
                    v.tensor_sub(wtr, t1, t2)
                    v.tensor_copy(wti, acc)
                v.tensor_copy(LC["dre"][:], wtr)
                v.tensor_scalar(LC["dim"][:], wti, neg1, None, op0=OP.mult)

                with tc.tile_pool(name=f"tf{l}", bufs=1) as ftp, \
                     tc.tile_pool(name=f"dblR{l}", bufs=1) as dwkR, \
                     tc.tile_pool(name=f"dblE{l}", bufs=1) as dwkE:
                    Rf_re = ftp.tile([128, NHB * T], F32, tag="Rf_re", name="Rf_re")
                    Rf_im = ftp.tile([128, NHB * T], F32, tag="Rf_im", name="Rf_im")
                    Ef_re = ftp.tile([128, NHB * T], F32, tag="Ef_re", name="Ef_re")
                    Ef_im = ftp.tile([128, NHB * T], F32, tag="Ef_im", name="Ef_im")
                    _build_table(gp, Rf_re, Rf_im, rpre, rpim, rpre, rpim, dwkR, NHB)
                    _build_table(v, Ef_re, Ef_im, esr, esi, wre, wim, dwkE, NHB)
                    sc.activation(LC["Rb_re"][:], Rf_re[:], AF.Copy)
                    sc.activation(LC["Rb_im"][:], Rf_im[:], AF.Copy)
                    sc.activation(LC["Eb_re"][:], Ef_re[:], AF.Copy)
                    sc.activation(LC["Eb_im"][:], Ef_im[:], AF.Copy)
            return LC

        # ---------- conv phase (H-shard): chunked-SSD convolution ----------
        def conv_phase(l, LC):
            Rb_re, Rb_im = LC["Rb_re"], LC["Rb_im"]
            Eb_re, Eb_im = LC["Eb_re"], LC["Eb_im"]
            drep = LC["drep"]
            with contextlib.ExitStack() as cv:
                stp = cv.enter_context(tc.tile_pool(name=f"st{l}", bufs=1))
                X_re = stp.tile([128, NHB * B * C], BF16, tag="X_re", name="X_re")
                X_im = stp.tile([128, NHB * B * C], BF16, tag="X_im", name="X_im")
                X_re4 = X_re[:].rearrange("p (h b c) -> p h b c", b=B, c=C)
                X_im4 = X_im[:].rearrange("p (h b c) -> p h b c", b=B, c=C)

                # collection: X[n, (b,c)] = R' @ u (transpose R' slices on the fly)
                with tc.tile_pool(name=f"wsl{l}", bufs=3) as wslp, \
                     tc.tile_pool(name=f"pst{l}", bufs=2, space="PSUM") as pstp, \
                     tc.tile_pool(name=f"psc{l}", bufs=2, space="PSUM") as pscp:
                    for k in range(NHB):
                        wsl = [wslp.tile([128, T], BF16, tag=f"wsl{comp}", name=f"wsl{comp}")
                               for comp in (0, 1)]
                        for comp, Rt in enumerate((Rb_re, Rb_im)):
                            psT = pstp.tile([128, T], BF16, tag="psT", name="psT")
                            te.transpose(psT[:], Rt[:, k * T : (k + 1) * T], identb[:])
                            sc.activation(wsl[comp][:], psT[:], AF.Copy)
                        psr = pscp.tile([128, B * C], F32, tag="psr", name="psr")
                        psi = pscp.tile([128, B * C], F32, tag="psi", name="psi")
                        for hp in (0, 1):
                            h = 2 * k + hp
                            us = u_slice(h)
                            te.matmul(psr[64 * hp : 64 * hp + 64, :],
                                      wsl[0][:, 64 * hp : 64 * hp + 64], us, start=True, stop=True)
                            te.matmul(psi[64 * hp : 64 * hp + 64, :],
                                      wsl[1][:, 64 * hp : 64 * hp + 64], us, start=True, stop=True)
                        sc.activation(X_re4[:, k, :, :], psr[:], AF.Copy)
                        sc.activation(X_im4[:, k, :, :], psi[:], AF.Copy)

                # scan (in place: X becomes state-history; fp32 ping-pong states)
                with tc.tile_pool(name=f"scan{l}", bufs=1) as sp:
                    halves = [("A", v, 0, HV), ("B", gp, HV, NHB)]
                    st = {}
                    for tag, eng, h0, h1 in halves:
                        nn = (h1 - h0) * B
                        for nm in ("Sr0", "Sr1", "Si0", "Si1", "tr", "ti", "w1", "w2"):
                            st[tag + nm] = sp.tile([128, nn], F32, tag=f"{nm}{tag}",
                                                   name=f"{nm}{tag}")[:].rearrange(
                                "p (h b) -> p h b", b=B)
                        nc.any.memset(st[tag + "Sr0"], 0.0)
                        nc.any.memset(st[tag + "Si0"], 0.0)
                    for ccc in range(C):
                        cur, nxt = str(ccc % 2), str((ccc + 1) % 2)
                        for tag, eng, h0, h1 in halves:
                            xr = X_re4[:, h0:h1, :, ccc]
                            xi = X_im4[:, h0:h1, :, ccc]
                            nh = h1 - h0
                            dreb = LC["dre"][:, h0:h1].unsqueeze(2).broadcast_to([128, nh, B])
                            dimb = LC["dim"][:, h0:h1].unsqueeze(2).broadcast_to([128, nh, B])
                            Src, Srn = st[tag + "Sr" + cur], st[tag + "Sr" + nxt]
                            Sic, Sin = st[tag + "Si" + cur], st[tag + "Si" + nxt]
                            tr, ti = st[tag + "tr"], st[tag + "ti"]
                            w1, w2 = st[tag + "w1"], st[tag + "w2"]
                            eng.tensor_add(tr, Src, xr)
                            eng.tensor_add(ti, Sic, xi)
                            eng.tensor_mul(w1, tr, dreb)
                            eng.tensor_mul(w2, ti, dimb)
                            eng.tensor_sub(Srn, w1, w2)
                            eng.tensor_mul(w1, ti, dreb)
                            eng.tensor_mul(w2, tr, dimb)
                            eng.tensor_add(Sin, w1, w2)
                            sc.activation(xr, Src, AF.Copy)
                            sc.activation(xi, Sic, AF.Copy)

                # per-h conv
                gt_pool = cv.enter_context(tc.tile_pool(name=f"gt{l}", bufs=3))
                ya_pool = cv.enter_context(tc.tile_pool(name=f"ya{l}", bufs=2))
                gp_ps = cv.enter_context(tc.tile_pool(name=f"gps{l}", bufs=2, space="PSUM"))
                cv_ps = cv.enter_context(tc.tile_pool(name=f"cvps{l}", bufs=2, space="PSUM"))
                HG = 8
                yg = [None, None]
                for h in range(HS):
                    hp, hb = h & 1, h >> 1
                    base = 64 * hp
                    er = Eb_re[base : base + 64, hb * T : (hb + 1) * T]
                    ei = Eb_im[base : base + 64, hb * T : (hb + 1) * T]
                    rr = Rb_re[base : base + 64, hb * T : (hb + 1) * T]
                    ri = Rb_im[base : base + 64, hb * T : (hb + 1) * T]
                    psG = gp_ps.tile([128, T], F32, tag="psG", name="psG")
                    te.matmul(psG[:], rr, er, start=True, stop=False)
                    te.matmul(psG[:], ri, ei, start=False, stop=True)
                    GTt = gt_pool.tile([128, T], BF16, tag="GTt", name="GTt")
                    GT = gt_pool.tile([128, T], BF16, tag="GT", name="GT")
                    v.tensor_mul(GTt[:], psG[:], trimask[:])
                    v.scalar_tensor_tensor(GT[:], ident[:], drep[:, h : h + 1], GTt[:],
                                           op0=OP.mult, op1=OP.add)
                    if h % HG == 0:
                        yg = [ya_pool.tile([128, HG * T], BF16, tag=f"yg{qq}", name=f"yg{qq}")
                              for qq in (0, 1)]
                    for qq in (0, 1):
                        ps = cv_ps.tile([128, T], F32, tag="ps", name="ps")
                        lu = u_slice(h, qq)
                        te.matmul(ps[:], lu, GT[:], start=True, stop=False)
                        lr = X_re[base : base + 64,
                                  hb * B * C + qq * 128 : hb * B * C + qq * 128 + 128]
                        li = X_im[base : base + 64,
                                  hb * B * C + qq * 128 : hb * B * C + qq * 128 + 128]
                        te.matmul(ps[:], lr, er, start=False, stop=False)
                        te.matmul(ps[:], li, ei, start=False, stop=True)
                        sc.activation(yg[qq][:, (h % HG) * T : (h % HG + 1) * T], ps[:],
                                      AF.Gelu_apprx_tanh)
                    if h % HG == HG - 1:
                        hg0 = h - HG + 1
                        for qq in (0, 1):
                            ygv = yg[qq][:].rearrange("bc (hh2 j) -> bc hh2 j", j=T)
                            for dd in range(4):
                                d = qq * 4 + dd
                                dst = a2a_y_in[l][d, hg0 : hg0 + HG, :, :].rearrange(
                                    "hh2 b2 (c j) -> (b2 c) hh2 j", j=T)
                                nc.sync.dma_start(dst, ygv[32 * dd : 32 * dd + 32, :, :])

        # ---------- GLU phase (B-shard): z[l, o] = y^T W, GLU, reshard ------
        def glu_phase(l):
            with contextlib.ExitStack() as gl:
                gpool = gl.enter_context(tc.tile_pool(name=f"glu{l}", bufs=1))
                wtiles = [gpool.tile([128, 2 * H], BF16, tag=f"wt{k}", name=f"wt{k}") for k in range(4)]
                ytiles = [gpool.tile([128, B2 * L], BF16, tag=f"yk{k}", name=f"yk{k}") for k in range(4)]
                for kt in range(4):
                    nc.sync.dma_start(wtiles[kt][:], par_in[("wt", l)][128 * kt : 128 * (kt + 1), :])
                    src = a2a_y_out[l][:].rearrange("s h b2 ll -> (s h) (b2 ll)")
                    nc.sync.dma_start(ytiles[kt][:], src[128 * kt : 128 * (kt + 1), :])
                if not bias_zero:
                    brow = gpool.tile([1, 2 * H], BF16, tag="brow", name="brow")
                    nc.sync.dma_start(brow[:], par_in[("brow", l)][:])
                zps = gl.enter_context(tc.tile_pool(name=f"zps{l}", bufs=4, space="PSUM"))
                zwp = gl.enter_context(tc.tile_pool(name=f"zw{l}", bufs=4))
                ubp = gl.enter_context(tc.tile_pool(name=f"ub{l}", bufs=3))
                zdt = BF16 if l == 0 else F32
                for lt in range(LT):
                    sl = slice(lt * T, (lt + 1) * T)
                    ps1 = zps.tile([128, H], F32, tag="ps1", name="ps1")
                    ps2 = zps.tile([128, H], F32, tag="ps2", name="ps2")
                    if not bias_zero:
                        te.matmul(ps1[:], onesb[:], brow[:, 0:H], start=True, stop=False)
                        te.matmul(ps2[:], onesb[:], brow[:, H : 2 * H], start=True, stop=False)
                    for kt in range(4):
                        te.matmul(ps1[:], ytiles[kt][:, sl], wtiles[kt][:, 0:H],
                                  start=(bias_zero and kt == 0), stop=(kt == 3))
                        te.matmul(ps2[:], ytiles[kt][:, sl], wtiles[kt][:, H : 2 * H],
                                  start=(bias_zero and kt == 0), stop=(kt == 3))
                    z1 = zwp.tile([128, H], zdt, tag="z1", name="z1")
                    sg = zwp.tile([128, H], zdt, tag="sg", name="sg")
                    sc.activation(sg[:], ps2[:], AF.Sigmoid)
                    v.tensor_mul(z1[:], ps1[:], sg[:])
                    b2c, ccc = lt // C, lt % C
                    if l == 0:
                        ub = ubp.tile([128, H], BF16, tag="ub", name="ub")
                        nc.sync.dma_start(ub[:], u0b_in[lt * T : (lt + 1) * T, :])
                        gp.tensor_add(z1[:], z1[:], ub[:])
                        for d in range(CORES):
                            nc.sync.dma_start(a2a_u_in[d, :, b2c, ccc, :],
                                              z1[:, HS * d : HS * (d + 1)])
                    else:
                        nc.sync.dma_start(out_z[b2c, ccc * T : (ccc + 1) * T, :], z1[:])
                if l == 0:
                    gp.collective_compute(
                        "AllToAll", OP.bypass, replica_groups=RG,
                        ins=[a2a_u_in[:].opt()], outs=[a2a_u_out[:].opt()])
                    for s in range(CORES):
                        src = a2a_u_out[s].rearrange("j b2 c h -> j (b2 c h)")
                        dstv = u_sb[:].rearrange("j (b c h) -> j b c h", b=B, c=C)[
                            :, 2 * s : 2 * s + 2, :, :].rearrange("j b c h -> j (b c h)")
                        nc.sync.dma_start(dstv, src)

        # ---------------- main flow ----------------
        ls0 = contextlib.ExitStack()
        LC0 = make_layer_consts(0, ls0)
        conv_phase(0, LC0)
        ls0.close()
        ls1 = contextlib.ExitStack()
        LC1 = make_layer_consts(1, ls1)   # overlaps the a2a_y0/GLU0 window
        gp.collective_compute(
            "AllToAll", OP.bypass, replica_groups=RG,
            ins=[a2a_y_in[0][:].opt()], outs=[a2a_y_out[0][:].opt()])
        glu_phase(0)
        conv_phase(1, LC1)
        ls1.close()
        gp.collective_compute(
            "AllToAll", OP.bypass, replica_groups=RG,
            ins=[a2a_y_in[1][:].opt()], outs=[a2a_y_out[1][:].opt()])
        glu_phase(1)
    nc.finalize()
    _NC_CACHE[key] = nc
    return nc


# ====================== host side ======================

def _prep_core_inputs(core, x, pars):
    hs = slice(HS * core, HS * (core + 1))
    ins = {}
    xs = x[:, :, hs]                                    # (B, L, 64)
    u0 = xs.reshape(B, C, T, HS).transpose(2, 0, 1, 3)  # (j, b, c, h)
    ins["u0"] = np.ascontiguousarray(u0).astype(bfloat16)
    xb = x[B2 * core : B2 * (core + 1)]                 # (2, L, H)
    ins["u0b"] = xb.reshape(B2 * L, H).astype(bfloat16)
    ins["trimask"] = np.triu(np.ones((T, T), np.float32))
    ins["ident"] = np.eye(T, dtype=np.float32)
    ins["identb"] = np.eye(T, dtype=np.float32).astype(bfloat16)

    def scan_layout(a):
        if a.ndim == 1:
            a = np.broadcast_to(a[:, None], (HS, N))
        return np.ascontiguousarray(
            a.reshape(NHB, 2, N).transpose(1, 2, 0).reshape(128, NHB)).astype(np.float32)

    for l in (0, 1):
        ins[f"ldt{l}"] = scan_layout(pars[f"ldt{l}"][hs])
        ins[f"lare{l}"] = scan_layout(pars[f"lAre{l}"][hs])
        ins[f"aim{l}"] = scan_layout(pars[f"Aim{l}"][hs])
        ins[f"cre{l}"] = scan_layout(pars[f"Cre{l}"][hs])
        ins[f"cim{l}"] = scan_layout(pars[f"Cim{l}"][hs])
        ins[f"drep{l}"] = np.ascontiguousarray(
            np.broadcast_to(pars[f"D{l}"][hs][None, :], (128, HS))).astype(np.float32)
        ins[f"wt{l}"] = np.ascontiguousarray(pars[f"W{l}"].T).astype(bfloat16)
        ins[f"brow{l}"] = pars[f"b{l}"][None, :].astype(bfloat16)
    return ins


def run(x, pars, debug=False, trace=False):
    bias_zero = all(not pars[f"b{l}"].any() for l in (0, 1))
    nc = build_kernel(bias_zero=bias_zero)
    in_maps = [_prep_core_inputs(c, x, pars) for c in range(CORES)]
    r = run_bass_kernel_spmd(nc, in_maps, core_ids=list(range(CORES)), trace=trace)
    outs = np.stack([r.results[c]["out"] for c in range(CORES)])  # (8, 2, L, H)
    full = outs.reshape(B, L, H)
    return full, r


def kernel(**inputs):
    x = np.asarray(inputs["x"], dtype=np.float32)
    pars = {k: np.asarray(vv, dtype=np.float32) for k, vv in inputs.items() if k != "x"}
    full, _ = run(x, pars)
    return full


# revision 36
# speedup vs baseline: 2.1982x; 1.1027x over previous
"""Trainium2 Bass kernel for the 2-layer S4D block (nn_MetaS4History).

Strategy (8 cores, single launch):
  - Conv phases H-sharded (64 channels/core, full batch): chunked-SSD convolution
    with T=128 chunks: per-h matmuls (G-build, intra, injection) + a 16-step
    scan for inter-chunk states, split across Vector/GpSimd with ping-pong
    state buffers.
  - GLU projections B-sharded (2 batch elems/core, full H): y-stationary
    W-moving matmuls producing [l, o] tiles directly (no output transposes).
  - Phase boundaries resharded with AllToAll collectives (3 total), bf16
    payloads. Layer-1 parameter math + power tables are computed during the
    layer-0 collective/GLU window so they stay off the critical path.
All matmul operands in bf16 (1 cyc/row vs 4 for fp32, fast weight load);
parameter math, scan state and PSUM accumulation stay fp32.
"""
import contextlib
import math
import numpy as np
from ml_dtypes import bfloat16
import concourse.bacc as bacc
import concourse.mybir as mybir
from concourse.tile import TileContext
from concourse.bass_utils import run_bass_kernel_spmd

F32 = mybir.dt.float32
BF16 = mybir.dt.bfloat16
AF = mybir.ActivationFunctionType
OP = mybir.AluOpType

CORES = 8
B, L, H, N = 16, 2048, 512, 64
T, C = 128, 16          # chunk len, chunk count
HS = H // CORES         # 64 channels per core
B2 = B // CORES         # 2 batch per core (GLU phase)
NHB = HS // 2           # 32 h-blocks (h = 2*hblk + hpar)
LT = B2 * L // T        # 32 row-chunks in GLU phase
HV = 20                 # h-blocks scanned on Vector (rest on GpSimd)
RG = [list(range(CORES))]

_NC_CACHE = {}


def _build_table(eng, t3re, t3im, seed_re, seed_im, mul_re, mul_im, wk, nhb):
    """Power table via doubling: tab[:, hb, j] = seed * mul^j, j in 0..T-1.
    t3re/t3im: [128, nhb, T] APs; seed/mul: [128, nhb] APs (sliced); wk: pool."""
    eng.tensor_copy(t3re[:, :, 0:1], seed_re.unsqueeze(2))
    eng.tensor_copy(t3im[:, :, 0:1], seed_im.unsqueeze(2))
    mre = wk.tile([128, nhb], F32, tag="dbl_mre", name="dbl_mre")
    mim = wk.tile([128, nhb], F32, tag="dbl_mim", name="dbl_mim")
    q1 = wk.tile([128, nhb], F32, tag="dbl_q1", name="dbl_q1")
    q2 = wk.tile([128, nhb], F32, tag="dbl_q2", name="dbl_q2")
    sc1 = wk.tile([128, nhb * T // 2], F32, tag="dbl_s1", name="dbl_s1")
    eng.tensor_copy(mre[:], mul_re)
    eng.tensor_copy(mim[:], mul_im)
    m = 1
    while m < T:
        mbre = mre[:].unsqueeze(2).broadcast_to([128, nhb, m])
        mbim = mim[:].unsqueeze(2).broadcast_to([128, nhb, m])
        s1 = sc1[:].rearrange("p (h j) -> p h j", j=T // 2)[:, :, 0:m]
        src_re, src_im = t3re[:, :, 0:m], t3im[:, :, 0:m]
        dst_re, dst_im = t3re[:, :, m : 2 * m], t3im[:, :, m : 2 * m]
        eng.tensor_mul(s1, src_im, mbim)
        eng.tensor_mul(dst_re, src_re, mbre)
        eng.tensor_sub(dst_re, dst_re, s1)
        eng.tensor_mul(s1, src_im, mbre)
        eng.tensor_mul(dst_im, src_re, mbim)
        eng.tensor_add(dst_im, dst_im, s1)
        m *= 2
        if m < T:
            eng.tensor_mul(q1[:], mre[:], mre[:])
            eng.tensor_mul(q2[:], mim[:], mim[:])
            eng.tensor_mul(mim[:], mre[:], mim[:])
            eng.tensor_add(mim[:], mim[:], mim[:])
            eng.tensor_sub(mre[:], q1[:], q2[:])


def build_kernel(bias_zero=True):
    key = bias_zero
    if key in _NC_CACHE:
        return _NC_CACHE[key]
    nc = bacc.Bacc(num_devices=CORES)
    v = nc.vector
    gp = nc.gpsimd
    sc = nc.scalar
    te = nc.tensor

    # ---------------- DRAM I/O ----------------
    u0_in = nc.dram_tensor("u0", [T, B, C, HS], BF16, kind="ExternalInput")
    u0b_in = nc.dram_tensor("u0b", [B2 * L, H], BF16, kind="ExternalInput")
    trimask_in = nc.dram_tensor("trimask", [T, T], F32, kind="ExternalInput")
    ident_in = nc.dram_tensor("ident", [T, T], F32, kind="ExternalInput")
    identb_in = nc.dram_tensor("identb", [T, T], BF16, kind="ExternalInput")
    par_in = {}
    for l in (0, 1):
        for nm in ("ldt", "lare", "aim", "cre", "cim"):
            par_in[(nm, l)] = nc.dram_tensor(f"{nm}{l}", [128, NHB], F32, kind="ExternalInput")
        par_in[("drep", l)] = nc.dram_tensor(f"drep{l}", [128, HS], F32, kind="ExternalInput")
        par_in[("wt", l)] = nc.dram_tensor(f"wt{l}", [H, 2 * H], BF16, kind="ExternalInput")
        par_in[("brow", l)] = nc.dram_tensor(f"brow{l}", [1, 2 * H], BF16, kind="ExternalInput")

    a2a_y_in = [nc.dram_tensor(f"a2aY{l}_in", [CORES, HS, B2, L], BF16) for l in (0, 1)]
    a2a_y_out = [nc.dram_tensor(f"a2aY{l}_out", [CORES, HS, B2, L], BF16) for l in (0, 1)]
    a2a_u_in = nc.dram_tensor("a2aU_in", [CORES, B2, T, HS, C], BF16)
    a2a_u_out = nc.dram_tensor("a2aU_out", [CORES, B2, T, HS, C], BF16)
    out_z = nc.dram_tensor("out", [B2, L, H], F32, kind="ExternalOutput")

    with TileContext(nc) as tc, contextlib.ExitStack() as top:
        cpool = top.enter_context(tc.tile_pool(name="consts", bufs=1))
        trimask = cpool.tile([T, T], F32, tag="trimask", name="trimask")
        ident = cpool.tile([T, T], F32, tag="ident", name="ident")
        identb = cpool.tile([T, T], BF16, tag="identb", name="identb")
        nc.sync.dma_start(trimask[:], trimask_in[:])
        nc.sync.dma_start(ident[:], ident_in[:])
        nc.sync.dma_start(identb[:], identb_in[:])
        csts = cpool.tile([128, 32], F32, tag="csts", name="csts")
        SINC = [1.0, -1.0 / 6, 1.0 / 120, -1.0 / 5040, 1.0 / 362880, -1.0 / 39916800]
        COSC = [1.0, -1.0 / 2, 1.0 / 24, -1.0 / 720, 1.0 / 40320, -1.0 / 3628800]
        for k in range(6):
            nc.any.memset(csts[:, k : k + 1], SINC[k])
            nc.any.memset(csts[:, 6 + k : 7 + k], COSC[k])
        nc.any.memset(csts[:, 12:13], -1.0)
        nc.any.memset(csts[:, 13:14], 2.0)
        nc.any.memset(csts[:, 14:15], 1.0 / 16)
        for k in range(11):
            nc.any.memset(csts[:, 16 + k : 17 + k], 1.0 / math.factorial(k))
        nc.any.memset(csts[:, 27:28], 1.0 / 8)
        onesb = cpool.tile([1, T], BF16, tag="onesb", name="onesb")
        nc.any.memset(onesb[:], 1.0)

        upool = top.enter_context(tc.tile_pool(name="u", bufs=1))
        u_sb = upool.tile([T, B * C * HS], BF16, tag="u_sb", name="u_sb")  # [j,(b,c,h)]
        nc.sync.dma_start(u_sb[:], u0_in[:].rearrange("j b c h -> j (b c h)"))

        def u_slice(h, bq=None):
            b0, nb = (0, B) if bq is None else (bq * 8, 8)
            ap = u_sb[:].rearrange("j (b c h) -> j b c h", b=B, c=C)
            return ap[:, b0 : b0 + nb, :, h]

        neg1 = csts[:, 12:13]
        two = csts[:, 13:14]
        s16 = csts[:, 14:15]

        # ---------- per-layer constants: params, D-rep, bf16 power tables ----
        def make_layer_consts(l, stack):
            lcp = stack.enter_context(tc.tile_pool(name=f"lc{l}", bufs=1))
            LC = {"drep": lcp.tile([128, HS], F32, tag="p_drep", name="p_drep")}
            nc.sync.dma_start(LC["drep"][:], par_in[("drep", l)][:])
            for nm in ("Rb_re", "Rb_im", "Eb_re", "Eb_im"):
                LC[nm] = lcp.tile([128, NHB * T], BF16, tag=nm, name=nm)
            LC["dre"] = lcp.tile([128, NHB], F32, tag="dreS", name="dreS")
            LC["dim"] = lcp.tile([128, NHB], F32, tag="dimS", name="dimS")

            with tc.tile_pool(name=f"par{l}", bufs=1) as pp:
                P = {}
                for nm in ("ldt", "lare", "aim", "cre", "cim"):
                    P[nm] = pp.tile([128, NHB], F32, tag=f"p_{nm}", name=f"p_{nm}")
                    nc.sync.dma_start(P[nm][:], par_in[(nm, l)][:])

                def wk(tag):
                    return pp.tile([128, NHB], F32, tag=tag, name=tag)[:]

                def exp_poly(out, x):
                    """out = e^x via (T10(x/8))^8; |x| <= 8. Accurate to ~1e-7."""
                    ea = wk("exp_a")
                    et = wk("exp_t")
                    v.tensor_scalar(ea, x, csts[:, 27:28], None, op0=OP.mult)  # y = x/8
                    v.tensor_scalar(et, ea, csts[:, 26:27], csts[:, 25:26], op0=OP.mult, op1=OP.add)
                    for k in range(8, -1, -1):
                        v.tensor_mul(et, et, ea)
                        v.tensor_scalar(et, et, csts[:, 16 + k : 17 + k], None, op0=OP.add)
                    for _ in range(3):
                        v.tensor_mul(et, et, et)
                    v.tensor_copy(out, et)

                dt, eA = wk("dt"), wk("eA")
                exp_poly(dt, P["ldt"][:])
                exp_poly(eA, P["lare"][:])
                dtAre, dtAim = wk("dtAre"), wk("dtAim")
                v.scalar_tensor_tensor(dtAre, dt, -1.0, eA, op0=OP.mult, op1=OP.mult)
                v.tensor_mul(dtAim, dt, P["aim"][:])
                mag = wk("mag")
                exp_poly(mag, dtAre)
                q, x2 = wk("q"), wk("x2")
                v.tensor_scalar(q, dtAim, s16, None, op0=OP.mult)
                v.tensor_mul(x2, q, q)
                acc, t1, t2 = wk("acc"), wk("t1"), wk("t2")
                cr, ci = wk("cr"), wk("ci")
                v.tensor_scalar(acc, x2, csts[:, 5:6], csts[:, 4:5], op0=OP.mult, op1=OP.add)
                for k in (3, 2, 1, 0):
                    v.tensor_mul(t1, acc, x2)
                    v.tensor_scalar(acc, t1, csts[:, k : k + 1], None, op0=OP.add)
                v.tensor_mul(ci, acc, q)
                v.tensor_scalar(acc, x2, csts[:, 11:12], csts[:, 10:11], op0=OP.mult, op1=OP.add)
                for k in (9, 8, 7, 6):
                    v.tensor_mul(t1, acc, x2)
                    v.tensor_scalar(acc, t1, csts[:, k : k + 1], None, op0=OP.add)
                v.tensor_copy(cr, acc)
                for _ in range(4):
                    v.tensor_mul(t1, cr, cr)
                    v.tensor_mul(t2, ci, ci)
                    v.scalar_tensor_tensor(acc, cr, 2.0, ci, op0=OP.mult, op1=OP.mult)
                    v.tensor_sub(cr, t1, t2)
                    v.tensor_copy(ci, acc)
                wre, wim = wk("wre"), wk("wim")
                v.tensor_mul(wre, mag, cr)
                v.tensor_mul(wim, mag, ci)
                m2, im2 = wk("m2"), wk("im2")
                v.tensor_mul(m2, mag, mag)
                v.reciprocal(im2, m2)
                rpre, rpim = wk("rpre"), wk("rpim")
                v.tensor_mul(rpre, wre, im2)
                v.tensor_mul(rpim, wim, im2)
                wm1re = wk("wm1re")
                v.tensor_scalar(wm1re, wre, neg1, None, op0=OP.add)
                tre, tim = wk("tre"), wk("tim")
                v.tensor_mul(t1, P["cre"][:], wm1re)
                v.tensor_mul(t2, P["cim"][:], wim)
                v.tensor_sub(tre, t1, t2)
                v.tensor_mul(t1, P["cre"][:], wim)
                v.tensor_mul(t2, P["cim"][:], wm1re)
                v.tensor_add(tim, t1, t2)
                den, invd = wk("den"), wk("invd")
                v.tensor_mul(t1, eA, eA)
                v.tensor_mul(t2, P["aim"][:], P["aim"][:])
                v.tensor_add(den, t1, t2)
                v.reciprocal(invd, den)
                ccr, cci = wk("ccr"), wk("cci")
                v.tensor_mul(t1, tre, eA)
                v.tensor_mul(t2, tim, P["aim"][:])
                v.tensor_sub(acc, t2, t1)
                v.tensor_mul(ccr, acc, invd)
                v.tensor_mul(t1, tre, P["aim"][:])
                v.tensor_mul(t2, tim, eA)
                v.tensor_add(acc, t1, t2)
                v.tensor_mul(t1, acc, invd)
                v.tensor_scalar(cci, t1, neg1, None, op0=OP.mult)
                esr, esi = wk("esr"), wk("esi")
                v.tensor_mul(t1, ccr, wre)
                v.tensor_mul(t2, cci, wim)
                v.tensor_sub(acc, t1, t2)
                v.tensor_scalar(esr, acc, two, None, op0=OP.mult)
                v.tensor_mul(t1, ccr, wim)
                v.tensor_mul(t2, cci, wre)
                v.tensor_add(acc, t1, t2)
                v.tensor_scalar(esi, acc, two, None, op0=OP.mult)
                wtr, wti = wk("wtr"), wk("wti")
                v.tensor_copy(wtr, wre)
                v.tensor_copy(wti, wim)
                for _ in range(7):
                    v.tensor_mul(t1, wtr, wtr)
                    v.tensor_mul(t2, wti, wti)
                    v.scalar_tensor_tensor(acc, wtr, 2.0, wti, op0=OP.mult, op1=OP.mult)
                    v.tensor_sub(wtr, t1, t2)
                    v.tensor_copy(wti, acc)
                v.tensor_copy(LC["dre"][:], wtr)
                v.tensor_scalar(LC["dim"][:], wti, neg1, None, op0=OP.mult)

                with tc.tile_pool(name=f"tf{l}", bufs=1) as ftp, \
                     tc.tile_pool(name=f"dbl0{l}", bufs=1) as dwk0, \
                     tc.tile_pool(name=f"dbl1{l}", bufs=1) as dwk1:
                    Rf_re = ftp.tile([128, NHB * T], F32, tag="Rf_re", name="Rf_re")
                    Rf_im = ftp.tile([128, NHB * T], F32, tag="Rf_im", name="Rf_im")
                    Ef_re = ftp.tile([128, NHB * T], F32, tag="Ef_re", name="Ef_re")
                    Ef_im = ftp.tile([128, NHB * T], F32, tag="Ef_im", name="Ef_im")
                    R3r = Rf_re[:].rearrange("p (h j) -> p h j", j=T)
                    R3i = Rf_im[:].rearrange("p (h j) -> p h j", j=T)
                    E3r = Ef_re[:].rearrange("p (h j) -> p h j", j=T)
                    E3i = Ef_im[:].rearrange("p (h j) -> p h j", j=T)
                    _build_table(gp, R3r, R3i, rpre, rpim, rpre, rpim, dwk1, NHB)
                    _build_table(v, E3r, E3i, esr, esi, wre, wim, dwk0, NHB)
                    sc.activation(LC["Rb_re"][:], Rf_re[:], AF.Copy)
                    sc.activation(LC["Rb_im"][:], Rf_im[:], AF.Copy)
                    sc.activation(LC["Eb_re"][:], Ef_re[:], AF.Copy)
                    sc.activation(LC["Eb_im"][:], Ef_im[:], AF.Copy)
            return LC

        # ---------- conv phase (H-shard): chunked-SSD convolution ----------
        def conv_phase(l, LC):
            Rb_re, Rb_im = LC["Rb_re"], LC["Rb_im"]
            Eb_re, Eb_im = LC["Eb_re"], LC["Eb_im"]
            drep = LC["drep"]
            with contextlib.ExitStack() as cv:
                stp = cv.enter_context(tc.tile_pool(name=f"st{l}", bufs=1))
                X_re = stp.tile([128, NHB * B * C], BF16, tag="X_re", name="X_re")
                X_im = stp.tile([128, NHB * B * C], BF16, tag="X_im", name="X_im")
                X_re4 = X_re[:].rearrange("p (h b c) -> p h b c", b=B, c=C)
                X_im4 = X_im[:].rearrange("p (h b c) -> p h b c", b=B, c=C)

                # collection: X[n, (b,c)] = R' @ u (transpose R' slices on the fly)
                with tc.tile_pool(name=f"wsl{l}", bufs=3) as wslp, \
                     tc.tile_pool(name=f"pst{l}", bufs=2, space="PSUM") as pstp, \
                     tc.tile_pool(name=f"psc{l}", bufs=2, space="PSUM") as pscp:
                    for k in range(NHB):
                        wsl = [wslp.tile([128, T], BF16, tag=f"wsl{comp}", name=f"wsl{comp}")
                               for comp in (0, 1)]
                        for comp, Rt in enumerate((Rb_re, Rb_im)):
                            psT = pstp.tile([128, T], BF16, tag="psT", name="psT")
                            te.transpose(psT[:], Rt[:, k * T : (k + 1) * T], identb[:])
                            sc.activation(wsl[comp][:], psT[:], AF.Copy)
                        psr = pscp.tile([128, B * C], F32, tag="psr", name="psr")
                        psi = pscp.tile([128, B * C], F32, tag="psi", name="psi")
                        for hp in (0, 1):
                            h = 2 * k + hp
                            us = u_slice(h)
                            te.matmul(psr[64 * hp : 64 * hp + 64, :],
                                      wsl[0][:, 64 * hp : 64 * hp + 64], us, start=True, stop=True)
                            te.matmul(psi[64 * hp : 64 * hp + 64, :],
                                      wsl[1][:, 64 * hp : 64 * hp + 64], us, start=True, stop=True)
                        sc.activation(X_re4[:, k, :, :], psr[:], AF.Copy)
                        sc.activation(X_im4[:, k, :, :], psi[:], AF.Copy)

                # scan (in place: X becomes state-history; fp32 ping-pong states)
                with tc.tile_pool(name=f"scan{l}", bufs=1) as sp:
                    halves = [("A", v, 0, HV), ("B", gp, HV, NHB)]
                    st = {}
                    for tag, eng, h0, h1 in halves:
                        nn = (h1 - h0) * B
                        for nm in ("Sr0", "Sr1", "Si0", "Si1", "tr", "ti", "w1", "w2"):
                            st[tag + nm] = sp.tile([128, nn], F32, tag=f"{nm}{tag}",
                                                   name=f"{nm}{tag}")[:].rearrange(
                                "p (h b) -> p h b", b=B)
                        nc.any.memset(st[tag + "Sr0"], 0.0)
                        nc.any.memset(st[tag + "Si0"], 0.0)
                    for ccc in range(C):
                        cur, nxt = str(ccc % 2), str((ccc + 1) % 2)
                        for tag, eng, h0, h1 in halves:
                            xr = X_re4[:, h0:h1, :, ccc]
                            xi = X_im4[:, h0:h1, :, ccc]
                            nh = h1 - h0
                            dreb = LC["dre"][:, h0:h1].unsqueeze(2).broadcast_to([128, nh, B])
                            dimb = LC["dim"][:, h0:h1].unsqueeze(2).broadcast_to([128, nh, B])
                            Src, Srn = st[tag + "Sr" + cur], st[tag + "Sr" + nxt]
                            Sic, Sin = st[tag + "Si" + cur], st[tag + "Si" + nxt]
                            tr, ti = st[tag + "tr"], st[tag + "ti"]
                            w1, w2 = st[tag + "w1"], st[tag + "w2"]
                            eng.tensor_add(tr, Src, xr)
                            eng.tensor_add(ti, Sic, xi)
                            eng.tensor_mul(w1, tr, dreb)
                            eng.tensor_mul(w2, ti, dimb)
                            eng.tensor_sub(Srn, w1, w2)
                            eng.tensor_mul(w1, ti, dreb)
                            eng.tensor_mul(w2, tr, dimb)
                            eng.tensor_add(Sin, w1, w2)
                            sc.activation(xr, Src, AF.Copy)
                            sc.activation(xi, Sic, AF.Copy)

                # per-h conv
                gt_pool = cv.enter_context(tc.tile_pool(name=f"gt{l}", bufs=3))
                ya_pool = cv.enter_context(tc.tile_pool(name=f"ya{l}", bufs=2))
                gp_ps = cv.enter_context(tc.tile_pool(name=f"gps{l}", bufs=2, space="PSUM"))
                cv_ps = cv.enter_context(tc.tile_pool(name=f"cvps{l}", bufs=2, space="PSUM"))
                HG = 8
                yg = [None, None]
                for h in range(HS):
                    hp, hb = h & 1, h >> 1
                    base = 64 * hp
                    er = Eb_re[base : base + 64, hb * T : (hb + 1) * T]
                    ei = Eb_im[base : base + 64, hb * T : (hb + 1) * T]
                    rr = Rb_re[base : base + 64, hb * T : (hb + 1) * T]
                    ri = Rb_im[base : base + 64, hb * T : (hb + 1) * T]
                    psG = gp_ps.tile([128, T], F32, tag="psG", name="psG")
                    te.matmul(psG[:], rr, er, start=True, stop=False)
                    te.matmul(psG[:], ri, ei, start=False, stop=True)
                    GTt = gt_pool.tile([128, T], BF16, tag="GTt", name="GTt")
                    GT = gt_pool.tile([128, T], BF16, tag="GT", name="GT")
                    v.tensor_mul(GTt[:], psG[:], trimask[:])
                    v.scalar_tensor_tensor(GT[:], ident[:], drep[:, h : h + 1], GTt[:],
                                           op0=OP.mult, op1=OP.add)
                    if h % HG == 0:
                        yg = [ya_pool.tile([128, HG * T], BF16, tag=f"yg{qq}", name=f"yg{qq}")
                              for qq in (0, 1)]
                    for qq in (0, 1):
                        ps = cv_ps.tile([128, T], F32, tag="ps", name="ps")
                        lu = u_slice(h, qq)
                        te.matmul(ps[:], lu, GT[:], start=True, stop=False)
                        lr = X_re[base : base + 64,
                                  hb * B * C + qq * 128 : hb * B * C + qq * 128 + 128]
                        li = X_im[base : base + 64,
                                  hb * B * C + qq * 128 : hb * B * C + qq * 128 + 128]
                        te.matmul(ps[:], lr, er, start=False, stop=False)
                        te.matmul(ps[:], li, ei, start=False, stop=True)
                        sc.activation(yg[qq][:, (h % HG) * T : (h % HG + 1) * T], ps[:],
                                      AF.Gelu_apprx_tanh)
                    if h % HG == HG - 1:
                        hg0 = h - HG + 1
                        for qq in (0, 1):
                            ygv = yg[qq][:].rearrange("bc (hh2 j) -> bc hh2 j", j=T)
                            for dd in range(4):
                                d = qq * 4 + dd
                                dst = a2a_y_in[l][d, hg0 : hg0 + HG, :, :].rearrange(
                                    "hh2 b2 (c j) -> (b2 c) hh2 j", j=T)
                                nc.sync.dma_start(dst, ygv[32 * dd : 32 * dd + 32, :, :])

        # ---------- GLU phase (B-shard): z[l, o] = y^T W, GLU, reshard ------
        def glu_phase(l):
            with contextlib.ExitStack() as gl:
                gpool = gl.enter_context(tc.tile_pool(name=f"glu{l}", bufs=1))
                wtiles = [gpool.tile([128, 2 * H], BF16, tag=f"wt{k}", name=f"wt{k}") for k in range(4)]
                ytiles = [gpool.tile([128, B2 * L], BF16, tag=f"yk{k}", name=f"yk{k}") for k in range(4)]
                for kt in range(4):
                    nc.sync.dma_start(wtiles[kt][:], par_in[("wt", l)][128 * kt : 128 * (kt + 1), :])
                    src = a2a_y_out[l][:].rearrange("s h b2 ll -> (s h) (b2 ll)")
                    nc.sync.dma_start(ytiles[kt][:], src[128 * kt : 128 * (kt + 1), :])
                if not bias_zero:
                    brow = gpool.tile([1, 2 * H], BF16, tag="brow", name="brow")
                    nc.sync.dma_start(brow[:], par_in[("brow", l)][:])
                zps = gl.enter_context(tc.tile_pool(name=f"zps{l}", bufs=4, space="PSUM"))
                zwp = gl.enter_context(tc.tile_pool(name=f"zw{l}", bufs=4))
                ubp = gl.enter_context(tc.tile_pool(name=f"ub{l}", bufs=3))
                if l == 0:
                    # staging for u1 in (h, c)-major layout so the a2a_u DMAs
                    # move 2 KB runs per partition instead of 128 B
                    zstg = [gpool.tile([128, C * H], BF16, tag=f"zstg{b2}", name=f"zstg{b2}")
                            for b2 in range(B2)]
                for lt in range(LT):
                    sl = slice(lt * T, (lt + 1) * T)
                    ps1 = zps.tile([128, H], F32, tag="ps1", name="ps1")
                    ps2 = zps.tile([128, H], F32, tag="ps2", name="ps2")
                    if not bias_zero:
                        te.matmul(ps1[:], onesb[:], brow[:, 0:H], start=True, stop=False)
                        te.matmul(ps2[:], onesb[:], brow[:, H : 2 * H], start=True, stop=False)
                    for kt in range(4):
                        te.matmul(ps1[:], ytiles[kt][:, sl], wtiles[kt][:, 0:H],
                                  start=(bias_zero and kt == 0), stop=(kt == 3))
                        te.matmul(ps2[:], ytiles[kt][:, sl], wtiles[kt][:, H : 2 * H],
                                  start=(bias_zero and kt == 0), stop=(kt == 3))
                    sg = zwp.tile([128, H], BF16 if l == 0 else F32, tag="sg", name="sg")
                    sc.activation(sg[:], ps2[:], AF.Sigmoid)
                    b2c, ccc = lt // C, lt % C
                    if l == 0:
                        z1 = zwp.tile([128, H], BF16, tag="z1", name="z1")
                        v.tensor_mul(z1[:], ps1[:], sg[:])
                        ub = ubp.tile([128, H], BF16, tag="ub", name="ub")
                        nc.sync.dma_start(ub[:], u0b_in[lt * T : (lt + 1) * T, :])
                        zsl = zstg[b2c][:].rearrange("j (h c) -> j h c", c=C)[:, :, ccc]
                        gp.tensor_add(zsl, z1[:], ub[:])
                        if ccc == C - 1:
                            for d in range(CORES):
                                nc.sync.dma_start(
                                    a2a_u_in[d, b2c].rearrange("j h c -> j (h c)"),
                                    zstg[b2c][:, HS * C * d : HS * C * (d + 1)])
                    else:
                        z1 = zwp.tile([128, H], F32, tag="z1", name="z1")
                        v.tensor_mul(z1[:], ps1[:], sg[:])
                        nc.sync.dma_start(out_z[b2c, ccc * T : (ccc + 1) * T, :], z1[:])

        # ---------------- main flow ----------------
        ls0 = contextlib.ExitStack()
        LC0 = make_layer_consts(0, ls0)
        conv_phase(0, LC0)
        ls0.close()
        ls1 = contextlib.ExitStack()
        LC1 = make_layer_consts(1, ls1)   # overlaps the a2a_y0/GLU0 window
        gp.collective_compute(
            "AllToAll", OP.bypass, replica_groups=RG,
            ins=[a2a_y_in[0][:].opt()], outs=[a2a_y_out[0][:].opt()])
        glu_phase(0)
        gp.collective_compute(
            "AllToAll", OP.bypass, replica_groups=RG,
            ins=[a2a_u_in[:].opt()], outs=[a2a_u_out[:].opt()])
        # refill u for layer 1: DMA the (h,c)-major a2a payload into a temp,
        # then re-layout to (c,h)-minor on Vector/GpSimd.
        with tc.tile_pool(name="ut", bufs=1) as utp:
            u_tmp = utp.tile([T, B * HS * C], BF16, tag="u_tmp", name="u_tmp")
            ut4 = u_tmp[:].rearrange("j (b h c) -> j b h c", h=HS, c=C)
            for b2 in range(B2):
                for s in range(CORES):
                    dstv = ut4[:, 2 * s + b2, :, :].rearrange("j h c -> j (h c)")
                    nc.sync.dma_start(dstv,
                                      a2a_u_out[s, b2].rearrange("j h c -> j (h c)"))
            us4 = u_sb[:].rearrange("j (b c h) -> j b c h", b=B, c=C)
            src_sw = ut4.rearrange("j b h c -> j b c h")
            BSPL = 12
            v.tensor_copy(us4[:, 0:BSPL], src_sw[:, 0:BSPL])
            gp.tensor_copy(us4[:, BSPL:B], src_sw[:, BSPL:B])
        conv_phase(1, LC1)
        ls1.close()
        gp.collective_compute(
            "AllToAll", OP.bypass, replica_groups=RG,
            ins=[a2a_y_in[1][:].opt()], outs=[a2a_y_out[1][:].opt()])
        glu_phase(1)
    nc.finalize()
    _NC_CACHE[key] = nc
    return nc


# ====================== host side ======================

def _prep_core_inputs(core, x, pars):
    hs = slice(HS * core, HS * (core + 1))
    ins = {}
    xs = x[:, :, hs]                                    # (B, L, 64)
    u0 = xs.reshape(B, C, T, HS).transpose(2, 0, 1, 3)  # (j, b, c, h)
    ins["u0"] = np.ascontiguousarray(u0).astype(bfloat16)
    xb = x[B2 * core : B2 * (core + 1)]                 # (2, L, H)
    ins["u0b"] = xb.reshape(B2 * L, H).astype(bfloat16)
    ins["trimask"] = np.triu(np.ones((T, T), np.float32))
    ins["ident"] = np.eye(T, dtype=np.float32)
    ins["identb"] = np.eye(T, dtype=np.float32).astype(bfloat16)

    def scan_layout(a):
        if a.ndim == 1:
            a = np.broadcast_to(a[:, None], (HS, N))
        return np.ascontiguousarray(
            a.reshape(NHB, 2, N).transpose(1, 2, 0).reshape(128, NHB)).astype(np.float32)

    for l in (0, 1):
        ins[f"ldt{l}"] = scan_layout(pars[f"ldt{l}"][hs])
        ins[f"lare{l}"] = scan_layout(pars[f"lAre{l}"][hs])
        ins[f"aim{l}"] = scan_layout(pars[f"Aim{l}"][hs])
        ins[f"cre{l}"] = scan_layout(pars[f"Cre{l}"][hs])
        ins[f"cim{l}"] = scan_layout(pars[f"Cim{l}"][hs])
        ins[f"drep{l}"] = np.ascontiguousarray(
            np.broadcast_to(pars[f"D{l}"][hs][None, :], (128, HS))).astype(np.float32)
        ins[f"wt{l}"] = np.ascontiguousarray(pars[f"W{l}"].T).astype(bfloat16)
        ins[f"brow{l}"] = pars[f"b{l}"][None, :].astype(bfloat16)
    return ins


def run(x, pars, debug=False, trace=False):
    bias_zero = all(not pars[f"b{l}"].any() for l in (0, 1))
    nc = build_kernel(bias_zero=bias_zero)
    in_maps = [_prep_core_inputs(c, x, pars) for c in range(CORES)]
    r = run_bass_kernel_spmd(nc, in_maps, core_ids=list(range(CORES)), trace=trace)
    outs = np.stack([r.results[c]["out"] for c in range(CORES)])  # (8, 2, L, H)
    full = outs.reshape(B, L, H)
    return full, r


def kernel(**inputs):
    x = np.asarray(inputs["x"], dtype=np.float32)
    pars = {k: np.asarray(vv, dtype=np.float32) for k, vv in inputs.items() if k != "x"}
    full, _ = run(x, pars)
    return full
